# revision 14
# baseline (speedup 1.0000x reference)
"""Multi-head self-attention (GQA + RoPE, causal) on 8 Trainium2 cores.

Sharding: tensor-parallel across heads for QKV projection + attention
(each core owns 2 q-heads and their kv-head), then AllToAlls reshard
the attention output from head-shards to sequence-shards, and each core
computes the output projection for its 512 rows against the full Wo.
Host concatenates the row shards - no AllReduce anywhere.

The attention is split by head so communication overlaps compute:
  P1a: proj + rope + head-0 attention for all 8 row-macros
  A2A#0 (head-0 vo shards)  ||  P1b: head-1 attention
  A2A#1 (head-1 vo shards)  ||  P3a: out-proj partial over even heads
  P3b: accumulate odd heads + bias, write row shard

All matmuls run as float32r (full PE speed at N>=512, ~1e-4 rel error).
Attention is computed in transposed layout (logits^T = k^T-tiles @ q^T)
so no score transposes are needed; softmax denominators accumulate on
DVE/GpSimd and reduce across partitions with a ones-column matmul.
"""

import numpy as np

N_CORES = 8
B, S, DIM = 2, 2048, 2048
N_HEADS, N_KV_HEADS, HD = 16, 4, 128
ROWS = B * S                     # 4096
RPC = ROWS // N_CORES            # 512 rows per core / per macro
KT = DIM // 128                  # 16 K tiles for the projections
SCALE = float(1.0 / np.sqrt(HD))
NEG = -60000.0                   # pre-scale masked logit; exp(SCALE*NEG) == 0

_cache = {}


def _fix_multiwait(nc):
    """Split >capacity sync waits (this walrus allows 1/inst, 2/EventSem)."""
    import bass_rust

    n = 0
    for f in nc.m.functions:
        for bb in f.blocks:
            insts = bb.instructions
            new_list = []
            changed = False
            for inst in insts:
                si = inst.sync_info
                cap = 2 if isinstance(inst, bass_rust.InstEventSemaphore) else 1
                if si is not None and len(si.on_wait) > cap:
                    waits = list(si.on_wait)
                    keep, extra = waits[:cap], waits[cap:]
                    for j in range(0, len(extra), 2):
                        es = bass_rust.InstEventSemaphore(
                            engine=inst.engine, name=f"waitfix_{n}"
                        )
                        es.sync_info = bass_rust.SyncInfo(
                            on_wait=extra[j : j + 2], on_update=[]
                        )
                        nc.register_instruction(es)
                        new_list.append(es)
                        n += 1
                    inst.sync_info = bass_rust.SyncInfo(
                        on_wait=keep, on_update=list(si.on_update)
                    )
                    changed = True
                new_list.append(inst)
            if changed:
                insts[:] = new_list
    return n


def _build(causal):
    import concourse.bass as bass
    import concourse.tile as tile
    from concourse import mybir
    from concourse.bass import ts

    F32R = mybir.dt.float32r
    F32 = mybir.dt.float32

    nc = bass.Bass("TRN2", target_bir_lowering=False, debug=False,
                   num_devices=N_CORES)

    # --- DRAM I/O (per core) ---
    xT = nc.dram_tensor("xT", [DIM, ROWS], F32R, kind="ExternalInput").ap()
    wqkv = nc.dram_tensor("wqkv", [DIM, 512], F32R, kind="ExternalInput").ap()
    bias_mt = nc.dram_tensor("bias_mt", [128, 4], F32R, kind="ExternalInput").ap()
    cos2 = nc.dram_tensor("cos2", [128, S], F32R, kind="ExternalInput").ap()
    sin2 = nc.dram_tensor("sin2", [128, S], F32R, kind="ExternalInput").ap()
    mask_tri = nc.dram_tensor("mask_tri", [128, 128], F32, kind="ExternalInput").ap()
    ident = nc.dram_tensor("ident", [128, 64], F32, kind="ExternalInput").ap()
    ones_col = nc.dram_tensor("ones_col", [128, 1], F32R, kind="ExternalInput").ap()
    ones_row = nc.dram_tensor("ones_row", [1, 128], F32R, kind="ExternalInput").ap()
    wo = nc.dram_tensor("wo", [DIM, DIM], F32R, kind="ExternalInput").ap()
    bo_b = nc.dram_tensor("bo_b", [128, DIM], F32, kind="ExternalInput").ap()
    out_ap = nc.dram_tensor("out", [RPC, DIM], F32, kind="ExternalOutput").ap()

    with tile.TileContext(nc) as tc:
        with (
            nc.allow_low_precision(reason="f32r attention kernel"),
            tc.tile_pool(name="dram", bufs=1, space="DRAM") as dram,
            tc.tile_pool(name="consts", bufs=1) as consts,
        ):
            # per-head A2A buffers: chunk m = [128, 512] voT of macro m
            a2a_in = [dram.tile([N_CORES * 128, RPC], F32R, name=f"a2ai{h}")
                      for h in range(2)]
            a2a_out = [dram.tile([N_CORES * 128, RPC], F32R, name=f"a2ao{h}")
                       for h in range(2)]

            # --- constants ---
            bias_t = consts.tile([128, 4], F32R, tag="bias")
            nc.sync.dma_start(bias_t[:], bias_mt[:])
            cos_t = consts.tile([128, S], F32R, tag="cos")
            sin_t = consts.tile([128, S], F32R, tag="sin")
            nc.sync.dma_start(cos_t[:], cos2[:])
            nc.sync.dma_start(sin_t[:], sin2[:])
            mask_t = consts.tile([128, 128], F32, tag="mask")
            nc.sync.dma_start(mask_t[:], mask_tri[:])
            id_t = consts.tile([128, 64], F32, tag="ident")
            nc.sync.dma_start(id_t[:], ident[:])
            onc_t = consts.tile([128, 1], F32R, tag="onc")
            nc.sync.dma_start(onc_t[:], ones_col[:])
            onr_t = consts.tile([1, 128], F32R, tag="onr")
            nc.sync.dma_start(onr_t[:], ones_row[:])
            bo_t = consts.tile([128, DIM], F32, tag="bo")
            nc.sync.dma_start(bo_t[:], bo_b[:])

            with (
                tc.tile_pool(name="wqp", bufs=1) as wq_pool,
                tc.tile_pool(name="xs", bufs=1) as xs_pool,
                tc.tile_pool(name="zp", bufs=2, space="PSUM") as zp_pool,
                tc.tile_pool(name="z", bufs=2) as z_pool,
                tc.tile_pool(name="rt", bufs=4) as rt_pool,
                tc.tile_pool(name="qtr", bufs=2) as qtr_pool,
                tc.tile_pool(name="q1k", bufs=1) as q1k_pool,
                tc.tile_pool(name="kv", bufs=1) as kv_pool,
                tc.tile_pool(name="aux", bufs=1, space="PSUM") as aux_pool,
                tc.tile_pool(name="lg", bufs=2, space="PSUM") as lg_pool,
                tc.tile_pool(name="vo", bufs=2, space="PSUM") as vo_pool,
                tc.tile_pool(name="sm", bufs=1, space="PSUM") as sm_pool,
                tc.tile_pool(name="ex", bufs=3) as ex_pool,
                tc.tile_pool(name="fin", bufs=2) as fin_pool,
            ):
                wq_t = wq_pool.tile([128, KT * 512], F32R, tag="wq")
                nc.sync.dma_start(
                    wq_t[:].rearrange("p (t m) -> p t m", t=KT),
                    wqkv.rearrange("(t p) m -> p t m", p=128),
                )

                ktr = {}       # batch -> [128, S] rope'd K^T
                v_tiles = {}   # batch -> 16 x [128, 128] V tiles
                q1_tiles = {}  # macro -> head-1 q^T (kept for P1b)

                def attention(m, h, q_tile):
                    """Causal attention for macro m, local head h."""
                    bat, j = divmod(m, 4)
                    n_t = 4 * j + 4 if causal else 16
                    vo_ps = vo_pool.tile([128, RPC], F32, tag="vo",
                                         name=f"vo_{m}_{h}")
                    acc = fin_pool.tile([128, RPC], F32R, tag="acc",
                                        name=f"acc_{m}_{h}")
                    for t in range(n_t):
                        r0 = max(0, 128 * t - 512 * j) if causal else 0
                        lp = lg_pool.tile([128, RPC], F32, tag="lp",
                                          name=f"lp_{m}_{h}_{t}")
                        nc.tensor.matmul(
                            lp[:, r0:RPC],
                            ktr[bat][:, 128 * t : 128 * t + 128],
                            q_tile[:, r0:RPC],
                            start=True, stop=True,
                        )
                        if causal and t >= 4 * j:
                            nc.vector.tensor_add(
                                lp[:, r0 : r0 + 128],
                                lp[:, r0 : r0 + 128],
                                mask_t[:],
                            )
                        e = ex_pool.tile([128, RPC], F32R, tag="e",
                                         name=f"e_{m}_{h}_{t}")
                        nc.scalar.activation(
                            e[:, r0:RPC], lp[:, r0:RPC],
                            mybir.ActivationFunctionType.Exp,
                            scale=SCALE,
                        )
                        if t == 0:
                            nc.gpsimd.tensor_copy(acc[:], e[:])
                        elif t % 2 == 1:
                            nc.gpsimd.tensor_add(
                                acc[:, r0:RPC], acc[:, r0:RPC], e[:, r0:RPC]
                            )
                        else:
                            nc.vector.tensor_add(
                                acc[:, r0:RPC], acc[:, r0:RPC], e[:, r0:RPC]
                            )
                        nc.tensor.matmul(
                            vo_ps[:, r0:RPC],
                            v_tiles[bat][t][:],
                            e[:, r0:RPC],
                            start=(t == 0),
                            stop=(t == n_t - 1),
                        )
                    # softmax denominators; normalize; ship to A2A buffer
                    sp = sm_pool.tile([1, RPC], F32, tag="sp",
                                      name=f"sp_{m}_{h}")
                    nc.tensor.matmul(sp[:], onc_t[:], acc[:],
                                     start=True, stop=True)
                    rc = fin_pool.tile([1, RPC], F32R, tag="rc",
                                       name=f"rc_{m}_{h}")
                    nc.vector.reciprocal(rc[:], sp[:])
                    bc = aux_pool.tile([128, RPC], F32, tag="aux",
                                       name=f"bc_{m}_{h}")
                    nc.tensor.matmul(bc[:], onr_t[:], rc[:],
                                     start=True, stop=True)
                    rcb = fin_pool.tile([128, RPC], F32, tag="rcb",
                                        name=f"rcb_{m}_{h}")
                    nc.scalar.copy(rcb[:], bc[:])
                    voT = fin_pool.tile([128, RPC], F32R, tag="voT",
                                        name=f"voT_{m}_{h}")
                    nc.vector.tensor_mul(voT[:], vo_ps[:], rcb[:])
                    nc.sync.dma_start(
                        a2a_in[h][128 * m : 128 * m + 128, :], voT[:]
                    )

                # ---- P1a: proj + rope + head-0 attention ----
                for m in range(N_CORES):
                    bat, j = divmod(m, 4)
                    r0_glob = m * RPC
                    if j == 0:
                        ktr[bat] = kv_pool.tile([128, S], F32R,
                                                tag=f"ktr{bat}",
                                                name=f"ktr_{bat}")
                        v_tiles[bat] = [
                            kv_pool.tile([128, 128], F32R, tag=f"v{bat}_{i}",
                                         name=f"v_{bat}_{i}")
                            for i in range(16)
                        ]

                    # projection: z^T tiles for this macro
                    xts = []
                    for k in range(KT):
                        xt = xs_pool.tile([128, RPC], F32R, tag=f"x{k}",
                                          name=f"xt_{m}_{k}")
                        nc.sync.dma_start(
                            xt[:], xT[ts(k, 128), r0_glob : r0_glob + RPC]
                        )
                        xts.append(xt)
                    z = []
                    for M in range(4):
                        zp = zp_pool.tile([128, RPC], F32, tag="zp",
                                          name=f"zp_{m}_{M}")
                        for k in range(KT):
                            nc.tensor.matmul(
                                zp[:],
                                wq_t[:, (k * 4 + M) * 128 : (k * 4 + M + 1) * 128],
                                xts[k][:],
                                start=(k == 0),
                                stop=(k == KT - 1),
                            )
                        zt = z_pool.tile([128, RPC], F32, tag=f"z{M}",
                                         name=f"z_{m}_{M}")
                        nc.scalar.activation(
                            zt[:], zp[:],
                            mybir.ActivationFunctionType.Identity,
                            bias=bias_t[:, M : M + 1],
                        )
                        z.append(zt)

                    # rope
                    sj = slice(512 * j, 512 * j + 512)
                    cs, sn = cos_t[:, sj], sin_t[:, sj]

                    def rope_half(dst, src_f, src_s, c_ap, s_ap, sign_f, nm):
                        t1 = rt_pool.tile([64, RPC], F32, tag="r1",
                                          name=f"r1_{nm}")
                        t2 = rt_pool.tile([64, RPC], F32, tag="r2",
                                          name=f"r2_{nm}")
                        nc.vector.tensor_mul(t1[:], src_f, c_ap)
                        nc.vector.tensor_mul(t2[:], src_s, s_ap)
                        if sign_f:
                            nc.vector.tensor_sub(dst, t1[:], t2[:])
                        else:
                            nc.vector.tensor_add(dst, t1[:], t2[:])

                    q_tr = []
                    for h in range(2):
                        if h == 0:
                            qt = qtr_pool.tile([128, RPC], F32R, tag="q0",
                                               name=f"q0_{m}")
                        else:
                            qt = q1k_pool.tile([128, RPC], F32R, tag=f"q1_{m}",
                                               name=f"q1_{m}")
                        hs = slice(64 * h, 64 * h + 64)
                        rope_half(qt[0:64, :], z[0][hs, :], z[1][hs, :],
                                  cs[hs, :], sn[hs, :], True, f"qf{m}{h}")
                        rope_half(qt[64:128, :], z[0][hs, :], z[1][hs, :],
                                  sn[hs, :], cs[hs, :], False, f"qs{m}{h}")
                        q_tr.append(qt)
                    q1_tiles[m] = q_tr[1]
                    rope_half(ktr[bat][0:64, sj], z[2][0:64, :], z[3][0:64, :],
                              cs[0:64, :], sn[0:64, :], True, f"kf{m}")
                    rope_half(ktr[bat][64:128, sj], z[2][0:64, :], z[3][0:64, :],
                              sn[0:64, :], cs[0:64, :], False, f"ks{m}")

                    # v transposes: z[2]/z[3] partitions 64:128 hold v halves
                    for t4 in range(4):
                        vp = aux_pool.tile([128, 512], F32, tag="aux",
                                           name=f"vp_{m}_{t4}")
                        cslice = slice(128 * t4, 128 * t4 + 128)
                        nc.tensor.transpose(
                            vp[:, 0:64], z[2][64:128, cslice], id_t[64:128, :]
                        )
                        nc.tensor.transpose(
                            vp[:, 64:128], z[3][64:128, cslice], id_t[64:128, :]
                        )
                        nc.scalar.copy(v_tiles[bat][4 * j + t4][:],
                                       vp[:, 0:128])

                    attention(m, 0, q_tr[0])

                # ---- A2A #0 || P1b: head-1 attention ----
                nc.gpsimd.collective_compute(
                    "AllToAll",
                    mybir.AluOpType.bypass,
                    replica_groups=[list(range(N_CORES))],
                    ins=[a2a_in[0].opt()],
                    outs=[a2a_out[0].opt()],
                )
                for m in range(N_CORES):
                    attention(m, 1, q1_tiles[m])
                nc.gpsimd.collective_compute(
                    "AllToAll",
                    mybir.AluOpType.bypass,
                    replica_groups=[list(range(N_CORES))],
                    ins=[a2a_in[1].opt()],
                    outs=[a2a_out[1].opt()],
                )

            # ---- P3: out-proj for my 512 rows (even heads, then odd) ----
            with (
                tc.tile_pool(name="vt", bufs=1) as vt_pool,
                tc.tile_pool(name="wop", bufs=1) as wo_pool,
                tc.tile_pool(name="op", bufs=2, space="PSUM") as op_pool,
                tc.tile_pool(name="ot", bufs=3) as ot_pool,
            ):
                wo_big = []
                for oc in range(4):
                    wb = wo_pool.tile([128, KT * 512], F32R, tag=f"wo{oc}",
                                      name=f"wo_{oc}")
                    nc.sync.dma_start(
                        wb[:].rearrange("p (t n) -> p t n", t=KT),
                        wo.rearrange("(t p) n -> p t n", p=128)[
                            :, :, ts(oc, 512)
                        ],
                    )
                    wo_big.append(wb)
                # voT_full K-tiles: parity h from a2a_out[h]; source core r
                # holds head 2r+h = Wo row-tile 2r+h
                vot_t = {}
                for h in range(2):
                    for r in range(8):
                        vt = vt_pool.tile([128, RPC], F32R, tag=f"vt{h}_{r}",
                                          name=f"vt_{h}_{r}")
                        nc.sync.dma_start(vt[:], a2a_out[h][ts(r, 128), :])
                        vot_t[2 * r + h] = vt
                for oc in range(4):
                    ps = [op_pool.tile([128, 512], F32, tag=f"op{M}",
                                       name=f"op_{oc}_{M}")
                          for M in range(4)]
                    for i, kk in enumerate(
                        [2 * r for r in range(8)] + [2 * r + 1 for r in range(8)]
                    ):
                        for M in range(4):
                            nc.tensor.matmul(
                                ps[M][:],
                                vot_t[kk][:, ts(M, 128)],
                                wo_big[oc][:, 512 * kk : 512 * kk + 512],
                                start=(i == 0),
                                stop=(i == KT - 1),
                            )
                    for M in range(4):
                        ot = ot_pool.tile([128, 512], F32, tag="ot",
                                          name=f"ot_{oc}_{M}")
                        nc.vector.tensor_add(
                            ot[:], ps[M][:], bo_t[:, ts(oc, 512)]
                        )
                        nc.sync.dma_start(
                            out_ap[ts(M, 128), ts(oc, 512)], ot[:]
                        )

    _fix_multiwait(nc)
    return nc


def _host_prep(x, cos, sin, Wq, bq, Wkv, bkv, Wo, bo):
    """Build the per-core input maps (all host-side slicing/transposes)."""
    xT = np.ascontiguousarray(x.reshape(ROWS, DIM).T)
    cosT = np.ascontiguousarray(cos.reshape(S, HD // 2).T)   # [64, S]
    sinT = np.ascontiguousarray(sin.reshape(S, HD // 2).T)
    cos2 = np.concatenate([cosT, cosT], axis=0)              # [128, S]
    sin2 = np.concatenate([sinT, sinT], axis=0)

    ii = np.arange(128)
    mask_tri = np.where(ii[None, :] >= ii[:, None], 0.0, NEG).astype(np.float32)
    ident = np.concatenate([np.zeros((64, 64), np.float32),
                            np.eye(64, dtype=np.float32)], axis=0)
    ones_col = np.ones((128, 1), np.float32)
    ones_row = np.ones((1, 128), np.float32)
    bo_b = np.broadcast_to(bo[None, :], (128, DIM)).copy()

    Wk, Wv = Wkv[:, : N_KV_HEADS * HD], Wkv[:, N_KV_HEADS * HD :]
    bk, bv = bkv[: N_KV_HEADS * HD], bkv[N_KV_HEADS * HD :]

    in_maps = []
    for c in range(N_CORES):
        h0, h1 = 2 * c, 2 * c + 1
        g = c // 2
        cols = [
            np.concatenate([Wq[:, h0 * HD : h0 * HD + 64],
                            Wq[:, h1 * HD : h1 * HD + 64]], axis=1),
            np.concatenate([Wq[:, h0 * HD + 64 : h0 * HD + 128],
                            Wq[:, h1 * HD + 64 : h1 * HD + 128]], axis=1),
            np.concatenate([Wk[:, g * HD : g * HD + 64],
                            Wv[:, g * HD : g * HD + 64]], axis=1),
            np.concatenate([Wk[:, g * HD + 64 : g * HD + 128],
                            Wv[:, g * HD + 64 : g * HD + 128]], axis=1),
        ]
        wqkv_c = np.ascontiguousarray(np.concatenate(cols, axis=1))
        bias_cols = [
            np.concatenate([bq[h0 * HD : h0 * HD + 64],
                            bq[h1 * HD : h1 * HD + 64]]),
            np.concatenate([bq[h0 * HD + 64 : h0 * HD + 128],
                            bq[h1 * HD + 64 : h1 * HD + 128]]),
            np.concatenate([bk[g * HD : g * HD + 64],
                            bv[g * HD : g * HD + 64]]),
            np.concatenate([bk[g * HD + 64 : g * HD + 128],
                            bv[g * HD + 64 : g * HD + 128]]),
        ]
        bias_mt = np.stack(bias_cols, axis=1).astype(np.float32)  # [128, 4]
        in_maps.append({
            "xT": xT, "wqkv": wqkv_c, "bias_mt": bias_mt,
            "cos2": cos2, "sin2": sin2, "mask_tri": mask_tri,
            "ident": ident, "ones_col": ones_col, "ones_row": ones_row,
            "wo": Wo, "bo_b": bo_b,
        })
    return in_maps


def kernel(x, cos, sin, Wq, bq, Wkv, bkv, Wo, bo, causal):
    from concourse.bass_utils import run_bass_kernel_spmd

    x = np.asarray(x, np.float32)
    cos = np.asarray(cos, np.float32)
    sin = np.asarray(sin, np.float32)
    Wq = np.asarray(Wq, np.float32)
    bq = np.asarray(bq, np.float32)
    Wkv = np.asarray(Wkv, np.float32)
    bkv = np.asarray(bkv, np.float32)
    Wo = np.asarray(Wo, np.float32)
    bo = np.asarray(bo, np.float32)
    causal = bool(np.asarray(causal).item())

    if causal not in _cache:
        _cache[causal] = _build(causal)
    nc = _cache[causal]

    in_maps = _host_prep(x, cos, sin, Wq, bq, Wkv, bkv, Wo, bo)
    res = run_bass_kernel_spmd(nc, in_maps, list(range(N_CORES)))
    out = np.concatenate([res.results[c]["out"] for c in range(N_CORES)], axis=0)
    return out.reshape(B, S, DIM)


# revision 19
# speedup vs baseline: 1.0023x; 1.0023x over previous
"""Multi-head self-attention (GQA + RoPE, causal) on 8 Trainium2 cores.

Sharding: tensor-parallel across heads for QKV projection + attention
(each core owns 2 q-heads and their kv-head), then AllToAlls reshard
the attention output from head-shards to sequence-shards, and each core
computes the output projection for its 512 rows against the full Wo.
Host concatenates the row shards - no AllReduce anywhere.

The attention is split by head so communication overlaps compute:
  P1a: proj + rope + head-0 attention for all 8 row-macros
  A2A#0 (head-0 vo shards)  ||  P1b: head-1 attention
  A2A#1 (head-1 vo shards)  ||  P3a: out-proj partial over even heads
  P3b: accumulate odd heads + bias, write row shard

All matmuls run as float32r (full PE speed at N>=512, ~1e-4 rel error).
Attention is computed in transposed layout (logits^T = k^T-tiles @ q^T)
so no score transposes are needed; softmax denominators accumulate on
DVE/GpSimd and reduce across partitions with a ones-column matmul.
"""

import numpy as np

N_CORES = 8
B, S, DIM = 2, 2048, 2048
N_HEADS, N_KV_HEADS, HD = 16, 4, 128
ROWS = B * S                     # 4096
RPC = ROWS // N_CORES            # 512 rows per core / per macro
KT = DIM // 128                  # 16 K tiles for the projections
SCALE = float(1.0 / np.sqrt(HD))
NEG = -60000.0                   # pre-scale masked logit; exp(SCALE*NEG) == 0

_cache = {}


def _fix_multiwait(nc):
    """Split >capacity sync waits (this walrus allows 1/inst, 2/EventSem)."""
    import bass_rust

    n = 0
    for f in nc.m.functions:
        for bb in f.blocks:
            insts = bb.instructions
            new_list = []
            changed = False
            for inst in insts:
                si = inst.sync_info
                cap = 2 if isinstance(inst, bass_rust.InstEventSemaphore) else 1
                if si is not None and len(si.on_wait) > cap:
                    waits = list(si.on_wait)
                    keep, extra = waits[:cap], waits[cap:]
                    for j in range(0, len(extra), 2):
                        es = bass_rust.InstEventSemaphore(
                            engine=inst.engine, name=f"waitfix_{n}"
                        )
                        es.sync_info = bass_rust.SyncInfo(
                            on_wait=extra[j : j + 2], on_update=[]
                        )
                        nc.register_instruction(es)
                        new_list.append(es)
                        n += 1
                    inst.sync_info = bass_rust.SyncInfo(
                        on_wait=keep, on_update=list(si.on_update)
                    )
                    changed = True
                new_list.append(inst)
            if changed:
                insts[:] = new_list
    return n


def _build(causal):
    import concourse.bass as bass
    import concourse.tile as tile
    from concourse import mybir
    from concourse.bass import ts

    F32R = mybir.dt.float32r
    F32 = mybir.dt.float32

    nc = bass.Bass("TRN2", target_bir_lowering=False, debug=False,
                   num_devices=N_CORES)

    # --- DRAM I/O (per core) ---
    xT = nc.dram_tensor("xT", [DIM, ROWS], F32R, kind="ExternalInput").ap()
    wqkv = nc.dram_tensor("wqkv", [DIM, 512], F32R, kind="ExternalInput").ap()
    bias_mt = nc.dram_tensor("bias_mt", [128, 4], F32R, kind="ExternalInput").ap()
    cos2 = nc.dram_tensor("cos2", [128, S], F32R, kind="ExternalInput").ap()
    sin2 = nc.dram_tensor("sin2", [128, S], F32R, kind="ExternalInput").ap()
    mask_tri = nc.dram_tensor("mask_tri", [128, 128], F32, kind="ExternalInput").ap()
    ident = nc.dram_tensor("ident", [128, 64], F32, kind="ExternalInput").ap()
    ones_col = nc.dram_tensor("ones_col", [128, 1], F32R, kind="ExternalInput").ap()
    ones_row = nc.dram_tensor("ones_row", [1, 128], F32R, kind="ExternalInput").ap()
    wo = nc.dram_tensor("wo", [DIM, DIM], F32R, kind="ExternalInput").ap()
    bo_b = nc.dram_tensor("bo_b", [128, DIM], F32, kind="ExternalInput").ap()
    out_ap = nc.dram_tensor("out", [RPC, DIM], F32, kind="ExternalOutput").ap()

    with tile.TileContext(nc) as tc:
        with (
            nc.allow_low_precision(reason="f32r attention kernel"),
            tc.tile_pool(name="dram", bufs=1, space="DRAM") as dram,
            tc.tile_pool(name="consts", bufs=1) as consts,
        ):
            # per-head A2A buffers: chunk m = [128, 512] voT of macro m
            a2a_in = [dram.tile([N_CORES * 128, RPC], F32R, name=f"a2ai{h}")
                      for h in range(2)]
            a2a_out = [dram.tile([N_CORES * 128, RPC], F32R, name=f"a2ao{h}")
                       for h in range(2)]

            # --- constants ---
            bias_t = consts.tile([128, 4], F32R, tag="bias")
            nc.sync.dma_start(bias_t[:], bias_mt[:])
            cos_t = consts.tile([128, S], F32R, tag="cos")
            sin_t = consts.tile([128, S], F32R, tag="sin")
            nc.sync.dma_start(cos_t[:], cos2[:])
            nc.sync.dma_start(sin_t[:], sin2[:])
            mask_t = consts.tile([128, 128], F32, tag="mask")
            nc.sync.dma_start(mask_t[:], mask_tri[:])
            id_t = consts.tile([128, 64], F32, tag="ident")
            nc.sync.dma_start(id_t[:], ident[:])
            onc_t = consts.tile([128, 1], F32R, tag="onc")
            nc.sync.dma_start(onc_t[:], ones_col[:])
            onr_t = consts.tile([1, 128], F32R, tag="onr")
            nc.sync.dma_start(onr_t[:], ones_row[:])
            bo_t = consts.tile([128, DIM], F32, tag="bo")
            nc.sync.dma_start(bo_t[:], bo_b[:])

            with (
                tc.tile_pool(name="wqp", bufs=1) as wq_pool,
                tc.tile_pool(name="xs", bufs=1) as xs_pool,
                tc.tile_pool(name="zp", bufs=2, space="PSUM") as zp_pool,
                tc.tile_pool(name="z", bufs=2) as z_pool,
                tc.tile_pool(name="rt", bufs=4) as rt_pool,
                tc.tile_pool(name="qtr", bufs=2) as qtr_pool,
                tc.tile_pool(name="q1k", bufs=1) as q1k_pool,
                tc.tile_pool(name="kv", bufs=1) as kv_pool,
                tc.tile_pool(name="aux", bufs=1, space="PSUM") as aux_pool,
                tc.tile_pool(name="lg", bufs=2, space="PSUM") as lg_pool,
                tc.tile_pool(name="vo", bufs=2, space="PSUM") as vo_pool,
                tc.tile_pool(name="sm", bufs=1, space="PSUM") as sm_pool,
                tc.tile_pool(name="ex", bufs=3) as ex_pool,
                tc.tile_pool(name="fin", bufs=2) as fin_pool,
            ):
                wq_tiles = []
                for k in range(KT):
                    wk = wq_pool.tile([128, 512], F32R, tag=f"wq{k}",
                                      name=f"wq_{k}")
                    nc.sync.dma_start(wk[:], wqkv[ts(k, 128), :])
                    wq_tiles.append(wk)

                q0_keep = {}
                ktr = {}       # batch -> [128, S] rope'd K^T
                v_tiles = {}   # batch -> 16 x [128, 128] V tiles
                q1_tiles = {}  # macro -> head-1 q^T (kept for P1b)

                def attention(m, h, q_tile):
                    """Causal attention for macro m, local head h."""
                    bat, j = divmod(m, 4)
                    n_t = 4 * j + 4 if causal else 16
                    vo_ps = vo_pool.tile([128, RPC], F32, tag="vo",
                                         name=f"vo_{m}_{h}")
                    acc = fin_pool.tile([128, RPC], F32R, tag="acc",
                                        name=f"acc_{m}_{h}")
                    for t in range(n_t):
                        r0 = max(0, 128 * t - 512 * j) if causal else 0
                        lp = lg_pool.tile([128, RPC], F32, tag="lp",
                                          name=f"lp_{m}_{h}_{t}")
                        nc.tensor.matmul(
                            lp[:, r0:RPC],
                            ktr[bat][:, 128 * t : 128 * t + 128],
                            q_tile[:, r0:RPC],
                            start=True, stop=True,
                        )
                        if causal and t >= 4 * j:
                            nc.vector.tensor_add(
                                lp[:, r0 : r0 + 128],
                                lp[:, r0 : r0 + 128],
                                mask_t[:],
                            )
                        e = ex_pool.tile([128, RPC], F32R, tag="e",
                                         name=f"e_{m}_{h}_{t}")
                        nc.scalar.activation(
                            e[:, r0:RPC], lp[:, r0:RPC],
                            mybir.ActivationFunctionType.Exp,
                            scale=SCALE,
                        )
                        if t == 0:
                            nc.gpsimd.tensor_copy(acc[:], e[:])
                        elif t % 2 == 1:
                            nc.gpsimd.tensor_add(
                                acc[:, r0:RPC], acc[:, r0:RPC], e[:, r0:RPC]
                            )
                        else:
                            nc.vector.tensor_add(
                                acc[:, r0:RPC], acc[:, r0:RPC], e[:, r0:RPC]
                            )
                        nc.tensor.matmul(
                            vo_ps[:, r0:RPC],
                            v_tiles[bat][t][:],
                            e[:, r0:RPC],
                            start=(t == 0),
                            stop=(t == n_t - 1),
                        )
                    # softmax denominators; normalize; ship to A2A buffer
                    sp = sm_pool.tile([1, RPC], F32, tag="sp",
                                      name=f"sp_{m}_{h}")
                    nc.tensor.matmul(sp[:], onc_t[:], acc[:],
                                     start=True, stop=True)
                    rc = fin_pool.tile([1, RPC], F32R, tag="rc",
                                       name=f"rc_{m}_{h}")
                    nc.vector.reciprocal(rc[:], sp[:])
                    bc = aux_pool.tile([128, RPC], F32, tag="aux",
                                       name=f"bc_{m}_{h}")
                    nc.tensor.matmul(bc[:], onr_t[:], rc[:],
                                     start=True, stop=True)
                    rcb = fin_pool.tile([128, RPC], F32, tag="rcb",
                                        name=f"rcb_{m}_{h}")
                    nc.scalar.copy(rcb[:], bc[:])
                    voT = fin_pool.tile([128, RPC], F32R, tag="voT",
                                        name=f"voT_{m}_{h}")
                    nc.vector.tensor_mul(voT[:], vo_ps[:], rcb[:])
                    nc.sync.dma_start(
                        a2a_in[h][128 * m : 128 * m + 128, :], voT[:]
                    )

                # ---- P1a: proj + rope + head-0 attention ----
                for m in range(N_CORES):
                    bat, j = divmod(m, 4)
                    r0_glob = m * RPC
                    if j == 0:
                        ktr[bat] = kv_pool.tile([128, S], F32R,
                                                tag=f"ktr{bat}",
                                                name=f"ktr_{bat}")
                        v_tiles[bat] = [
                            kv_pool.tile([128, 128], F32R, tag=f"v{bat}_{i}",
                                         name=f"v_{bat}_{i}")
                            for i in range(16)
                        ]

                    # projection: z^T tiles for this macro
                    xts = []
                    for k in range(KT):
                        xt = xs_pool.tile([128, RPC], F32R, tag=f"x{k}",
                                          name=f"xt_{m}_{k}")
                        nc.sync.dma_start(
                            xt[:], xT[ts(k, 128), r0_glob : r0_glob + RPC]
                        )
                        xts.append(xt)
                    z = []
                    for M in range(4):
                        zp = zp_pool.tile([128, RPC], F32, tag="zp",
                                          name=f"zp_{m}_{M}")
                        for k in range(KT):
                            nc.tensor.matmul(
                                zp[:],
                                wq_tiles[k][:, ts(M, 128)],
                                xts[k][:],
                                start=(k == 0),
                                stop=(k == KT - 1),
                            )
                        zt = z_pool.tile([128, RPC], F32, tag=f"z{M}",
                                         name=f"z_{m}_{M}")
                        nc.scalar.activation(
                            zt[:], zp[:],
                            mybir.ActivationFunctionType.Identity,
                            bias=bias_t[:, M : M + 1],
                        )
                        z.append(zt)

                    # rope
                    sj = slice(512 * j, 512 * j + 512)
                    cs, sn = cos_t[:, sj], sin_t[:, sj]

                    def rope_half(dst, src_f, src_s, c_ap, s_ap, sign_f, nm):
                        t1 = rt_pool.tile([64, RPC], F32, tag="r1",
                                          name=f"r1_{nm}")
                        t2 = rt_pool.tile([64, RPC], F32, tag="r2",
                                          name=f"r2_{nm}")
                        nc.vector.tensor_mul(t1[:], src_f, c_ap)
                        nc.vector.tensor_mul(t2[:], src_s, s_ap)
                        if sign_f:
                            nc.vector.tensor_sub(dst, t1[:], t2[:])
                        else:
                            nc.vector.tensor_add(dst, t1[:], t2[:])

                    q_tr = []
                    for h in range(2):
                        if h == 0:
                            if causal:
                                qt = qtr_pool.tile([128, RPC], F32R, tag="q0",
                                                   name=f"q0_{m}")
                            else:
                                qt = q1k_pool.tile([128, RPC], F32R,
                                                   tag=f"q0_{m}",
                                                   name=f"q0k_{m}")
                        else:
                            qt = q1k_pool.tile([128, RPC], F32R, tag=f"q1_{m}",
                                               name=f"q1_{m}")
                        hs = slice(64 * h, 64 * h + 64)
                        rope_half(qt[0:64, :], z[0][hs, :], z[1][hs, :],
                                  cs[hs, :], sn[hs, :], True, f"qf{m}{h}")
                        rope_half(qt[64:128, :], z[0][hs, :], z[1][hs, :],
                                  sn[hs, :], cs[hs, :], False, f"qs{m}{h}")
                        q_tr.append(qt)
                    q1_tiles[m] = q_tr[1]
                    rope_half(ktr[bat][0:64, sj], z[2][0:64, :], z[3][0:64, :],
                              cs[0:64, :], sn[0:64, :], True, f"kf{m}")
                    rope_half(ktr[bat][64:128, sj], z[2][0:64, :], z[3][0:64, :],
                              sn[0:64, :], cs[0:64, :], False, f"ks{m}")

                    # v transposes: z[2]/z[3] partitions 64:128 hold v halves
                    for t4 in range(4):
                        vp = aux_pool.tile([128, 512], F32, tag="aux",
                                           name=f"vp_{m}_{t4}")
                        cslice = slice(128 * t4, 128 * t4 + 128)
                        nc.tensor.transpose(
                            vp[:, 0:64], z[2][64:128, cslice], id_t[64:128, :]
                        )
                        nc.tensor.transpose(
                            vp[:, 64:128], z[3][64:128, cslice], id_t[64:128, :]
                        )
                        nc.scalar.copy(v_tiles[bat][4 * j + t4][:],
                                       vp[:, 0:128])

                    if causal:
                        attention(m, 0, q_tr[0])
                        q0_keep[m] = None
                    else:
                        # non-causal needs the whole batch's KV first
                        q0_keep[m] = q_tr[0]
                        if j == 3:
                            for mm in range(4 * bat, 4 * bat + 4):
                                attention(mm, 0, q0_keep[mm])
                                q0_keep[mm] = None

                # ---- A2A #0 || P1b: head-1 attention ----
                nc.gpsimd.collective_compute(
                    "AllToAll",
                    mybir.AluOpType.bypass,
                    replica_groups=[list(range(N_CORES))],
                    ins=[a2a_in[0].opt()],
                    outs=[a2a_out[0].opt()],
                )
                for m in range(N_CORES):
                    attention(m, 1, q1_tiles[m])
                nc.gpsimd.collective_compute(
                    "AllToAll",
                    mybir.AluOpType.bypass,
                    replica_groups=[list(range(N_CORES))],
                    ins=[a2a_in[1].opt()],
                    outs=[a2a_out[1].opt()],
                )

            # ---- P3: out-proj for my 512 rows (even heads, then odd) ----
            with (
                tc.tile_pool(name="vt", bufs=1) as vt_pool,
                tc.tile_pool(name="wop", bufs=1) as wo_pool,
                tc.tile_pool(name="op", bufs=2, space="PSUM") as op_pool,
                tc.tile_pool(name="ot", bufs=3) as ot_pool,
            ):
                wo_big = []
                for oc in range(4):
                    wb = wo_pool.tile([128, KT * 512], F32R, tag=f"wo{oc}",
                                      name=f"wo_{oc}")
                    nc.sync.dma_start(
                        wb[:].rearrange("p (t n) -> p t n", t=KT),
                        wo.rearrange("(t p) n -> p t n", p=128)[
                            :, :, ts(oc, 512)
                        ],
                    )
                    wo_big.append(wb)
                # voT_full K-tiles: parity h from a2a_out[h]; source core r
                # holds head 2r+h = Wo row-tile 2r+h
                vot_t = {}
                for h in range(2):
                    for r in range(8):
                        vt = vt_pool.tile([128, RPC], F32R, tag=f"vt{h}_{r}",
                                          name=f"vt_{h}_{r}")
                        nc.sync.dma_start(vt[:], a2a_out[h][ts(r, 128), :])
                        vot_t[2 * r + h] = vt
                for oc in range(4):
                    ps = [op_pool.tile([128, 512], F32, tag=f"op{M}",
                                       name=f"op_{oc}_{M}")
                          for M in range(4)]
                    for i, kk in enumerate(
                        [2 * r for r in range(8)] + [2 * r + 1 for r in range(8)]
                    ):
                        for M in range(4):
                            nc.tensor.matmul(
                                ps[M][:],
                                vot_t[kk][:, ts(M, 128)],
                                wo_big[oc][:, 512 * kk : 512 * kk + 512],
                                start=(i == 0),
                                stop=(i == KT - 1),
                            )
                    for M in range(4):
                        ot = ot_pool.tile([128, 512], F32, tag="ot",
                                          name=f"ot_{oc}_{M}")
                        nc.vector.tensor_add(
                            ot[:], ps[M][:], bo_t[:, ts(oc, 512)]
                        )
                        nc.sync.dma_start(
                            out_ap[ts(M, 128), ts(oc, 512)], ot[:]
                        )

    _fix_multiwait(nc)
    return nc


def _host_prep(x, cos, sin, Wq, bq, Wkv, bkv, Wo, bo):
    """Build the per-core input maps (all host-side slicing/transposes)."""
    xT = np.ascontiguousarray(x.reshape(ROWS, DIM).T)
    cosT = np.ascontiguousarray(cos.reshape(S, HD // 2).T)   # [64, S]
    sinT = np.ascontiguousarray(sin.reshape(S, HD // 2).T)
    cos2 = np.concatenate([cosT, cosT], axis=0)              # [128, S]
    sin2 = np.concatenate([sinT, sinT], axis=0)

    ii = np.arange(128)
    mask_tri = np.where(ii[None, :] >= ii[:, None], 0.0, NEG).astype(np.float32)
    ident = np.concatenate([np.zeros((64, 64), np.float32),
                            np.eye(64, dtype=np.float32)], axis=0)
    ones_col = np.ones((128, 1), np.float32)
    ones_row = np.ones((1, 128), np.float32)
    bo_b = np.broadcast_to(bo[None, :], (128, DIM)).copy()

    Wk, Wv = Wkv[:, : N_KV_HEADS * HD], Wkv[:, N_KV_HEADS * HD :]
    bk, bv = bkv[: N_KV_HEADS * HD], bkv[N_KV_HEADS * HD :]

    in_maps = []
    for c in range(N_CORES):
        h0, h1 = 2 * c, 2 * c + 1
        g = c // 2
        cols = [
            np.concatenate([Wq[:, h0 * HD : h0 * HD + 64],
                            Wq[:, h1 * HD : h1 * HD + 64]], axis=1),
            np.concatenate([Wq[:, h0 * HD + 64 : h0 * HD + 128],
                            Wq[:, h1 * HD + 64 : h1 * HD + 128]], axis=1),
            np.concatenate([Wk[:, g * HD : g * HD + 64],
                            Wv[:, g * HD : g * HD + 64]], axis=1),
            np.concatenate([Wk[:, g * HD + 64 : g * HD + 128],
                            Wv[:, g * HD + 64 : g * HD + 128]], axis=1),
        ]
        wqkv_c = np.ascontiguousarray(np.concatenate(cols, axis=1))
        bias_cols = [
            np.concatenate([bq[h0 * HD : h0 * HD + 64],
                            bq[h1 * HD : h1 * HD + 64]]),
            np.concatenate([bq[h0 * HD + 64 : h0 * HD + 128],
                            bq[h1 * HD + 64 : h1 * HD + 128]]),
            np.concatenate([bk[g * HD : g * HD + 64],
                            bv[g * HD : g * HD + 64]]),
            np.concatenate([bk[g * HD + 64 : g * HD + 128],
                            bv[g * HD + 64 : g * HD + 128]]),
        ]
        bias_mt = np.stack(bias_cols, axis=1).astype(np.float32)  # [128, 4]
        in_maps.append({
            "xT": xT, "wqkv": wqkv_c, "bias_mt": bias_mt,
            "cos2": cos2, "sin2": sin2, "mask_tri": mask_tri,
            "ident": ident, "ones_col": ones_col, "ones_row": ones_row,
            "wo": Wo, "bo_b": bo_b,
        })
    return in_maps


def kernel(x, cos, sin, Wq, bq, Wkv, bkv, Wo, bo, causal):
    from concourse.bass_utils import run_bass_kernel_spmd

    x = np.asarray(x, np.float32)
    cos = np.asarray(cos, np.float32)
    sin = np.asarray(sin, np.float32)
    Wq = np.asarray(Wq, np.float32)
    bq = np.asarray(bq, np.float32)
    Wkv = np.asarray(Wkv, np.float32)
    bkv = np.asarray(bkv, np.float32)
    Wo = np.asarray(Wo, np.float32)
    bo = np.asarray(bo, np.float32)
    causal = bool(np.asarray(causal).item())

    if causal not in _cache:
        _cache[causal] = _build(causal)
    nc = _cache[causal]

    in_maps = _host_prep(x, cos, sin, Wq, bq, Wkv, bkv, Wo, bo)
    res = run_bass_kernel_spmd(nc, in_maps, list(range(N_CORES)))
    out = np.concatenate([res.results[c]["out"] for c in range(N_CORES)], axis=0)
    return out.reshape(B, S, DIM)


# revision 20
# speedup vs baseline: 1.0088x; 1.0064x over previous
"""Multi-head self-attention (GQA + RoPE, causal) on 8 Trainium2 cores.

Sharding: tensor-parallel across heads for QKV projection + attention
(each core owns 2 q-heads and their kv-head), then AllToAlls reshard
the attention output from head-shards to sequence-shards, and each core
computes the output projection for its 512 rows against the full Wo.
Host concatenates the row shards - no AllReduce anywhere.

The attention is split by head so communication overlaps compute:
  P1a: proj + rope + head-0 attention for all 8 row-macros
  A2A#0 (head-0 vo shards)  ||  P1b: head-1 attention
  A2A#1 (head-1 vo shards)  ||  P3a: out-proj partial over even heads
  P3b: accumulate odd heads + bias, write row shard

All matmuls run as float32r (full PE speed at N>=512, ~1e-4 rel error).
Attention is computed in transposed layout (logits^T = k^T-tiles @ q^T)
so no score transposes are needed; softmax denominators accumulate on
DVE/GpSimd and reduce across partitions with a ones-column matmul.
"""

import numpy as np

N_CORES = 8
B, S, DIM = 2, 2048, 2048
N_HEADS, N_KV_HEADS, HD = 16, 4, 128
ROWS = B * S                     # 4096
RPC = ROWS // N_CORES            # 512 rows per core / per macro
KT = DIM // 128                  # 16 K tiles for the projections
SCALE = float(1.0 / np.sqrt(HD))
NEG = -60000.0                   # pre-scale masked logit; exp(SCALE*NEG) == 0

_cache = {}


def _fix_multiwait(nc):
    """Split >capacity sync waits (this walrus allows 1/inst, 2/EventSem)."""
    import bass_rust

    n = 0
    for f in nc.m.functions:
        for bb in f.blocks:
            insts = bb.instructions
            new_list = []
            changed = False
            for inst in insts:
                si = inst.sync_info
                cap = 2 if isinstance(inst, bass_rust.InstEventSemaphore) else 1
                if si is not None and len(si.on_wait) > cap:
                    waits = list(si.on_wait)
                    keep, extra = waits[:cap], waits[cap:]
                    for j in range(0, len(extra), 2):
                        es = bass_rust.InstEventSemaphore(
                            engine=inst.engine, name=f"waitfix_{n}"
                        )
                        es.sync_info = bass_rust.SyncInfo(
                            on_wait=extra[j : j + 2], on_update=[]
                        )
                        nc.register_instruction(es)
                        new_list.append(es)
                        n += 1
                    inst.sync_info = bass_rust.SyncInfo(
                        on_wait=keep, on_update=list(si.on_update)
                    )
                    changed = True
                new_list.append(inst)
            if changed:
                insts[:] = new_list
    return n


def _build(causal):
    import concourse.bass as bass
    import concourse.tile as tile
    from concourse import mybir
    from concourse.bass import ts

    F32R = mybir.dt.float32r
    F32 = mybir.dt.float32

    nc = bass.Bass("TRN2", target_bir_lowering=False, debug=False,
                   num_devices=N_CORES)

    # --- DRAM I/O (per core) ---
    xT = nc.dram_tensor("xT", [DIM, ROWS], F32R, kind="ExternalInput").ap()
    wqkv = nc.dram_tensor("wqkv", [DIM, 512], F32R, kind="ExternalInput").ap()
    bias_mt = nc.dram_tensor("bias_mt", [128, 4], F32R, kind="ExternalInput").ap()
    cos2 = nc.dram_tensor("cos2", [128, S], F32R, kind="ExternalInput").ap()
    sin2 = nc.dram_tensor("sin2", [128, S], F32R, kind="ExternalInput").ap()
    mask_tri = nc.dram_tensor("mask_tri", [128, 128], F32, kind="ExternalInput").ap()
    ident = nc.dram_tensor("ident", [128, 64], F32, kind="ExternalInput").ap()
    ones_col = nc.dram_tensor("ones_col", [128, 1], F32R, kind="ExternalInput").ap()
    ones_row = nc.dram_tensor("ones_row", [1, 128], F32R, kind="ExternalInput").ap()
    wo = nc.dram_tensor("wo", [DIM, DIM], F32R, kind="ExternalInput").ap()
    bo_b = nc.dram_tensor("bo_b", [128, DIM], F32, kind="ExternalInput").ap()
    out_ap = nc.dram_tensor("out", [RPC, DIM], F32, kind="ExternalOutput").ap()

    with tile.TileContext(nc) as tc:
        with (
            nc.allow_low_precision(reason="f32r attention kernel"),
            tc.tile_pool(name="dram", bufs=1, space="DRAM") as dram,
            tc.tile_pool(name="consts", bufs=1) as consts,
        ):
            # per-head A2A buffers: chunk m = [128, 512] voT of macro m
            a2a_in = [dram.tile([N_CORES * 128, RPC], F32R, name=f"a2ai{h}")
                      for h in range(2)]
            a2a_out = [dram.tile([N_CORES * 128, RPC], F32R, name=f"a2ao{h}")
                       for h in range(2)]

            # --- constants ---
            bias_t = consts.tile([128, 4], F32R, tag="bias")
            nc.sync.dma_start(bias_t[:], bias_mt[:])
            cos_t = consts.tile([128, S], F32R, tag="cos")
            sin_t = consts.tile([128, S], F32R, tag="sin")
            nc.sync.dma_start(cos_t[:], cos2[:])
            nc.sync.dma_start(sin_t[:], sin2[:])
            mask_t = consts.tile([128, 128], F32, tag="mask")
            nc.sync.dma_start(mask_t[:], mask_tri[:])
            id_t = consts.tile([128, 64], F32, tag="ident")
            nc.sync.dma_start(id_t[:], ident[:])
            onc_t = consts.tile([128, 1], F32R, tag="onc")
            nc.sync.dma_start(onc_t[:], ones_col[:])
            onr_t = consts.tile([1, 128], F32R, tag="onr")
            nc.sync.dma_start(onr_t[:], ones_row[:])
            bo_t = consts.tile([128, DIM], F32, tag="bo")

            with (
                tc.tile_pool(name="wqp", bufs=1) as wq_pool,
                tc.tile_pool(name="xs", bufs=1) as xs_pool,
                tc.tile_pool(name="zp", bufs=2, space="PSUM") as zp_pool,
                tc.tile_pool(name="z", bufs=2) as z_pool,
                tc.tile_pool(name="rt", bufs=4) as rt_pool,
                tc.tile_pool(name="qtr", bufs=2) as qtr_pool,
                tc.tile_pool(name="q1k", bufs=1) as q1k_pool,
                tc.tile_pool(name="kv", bufs=1) as kv_pool,
                tc.tile_pool(name="aux", bufs=1, space="PSUM") as aux_pool,
                tc.tile_pool(name="lg", bufs=2, space="PSUM") as lg_pool,
                tc.tile_pool(name="vo", bufs=2, space="PSUM") as vo_pool,
                tc.tile_pool(name="sm", bufs=1, space="PSUM") as sm_pool,
                tc.tile_pool(name="ex", bufs=3) as ex_pool,
                tc.tile_pool(name="fin", bufs=2) as fin_pool,
            ):
                wq_tiles = []
                for k in range(KT):
                    wk = wq_pool.tile([128, 512], F32R, tag=f"wq{k}",
                                      name=f"wq_{k}")
                    nc.sync.dma_start(wk[:], wqkv[ts(k, 128), :])
                    wq_tiles.append(wk)

                q0_keep = {}
                ktr = {}       # batch -> [128, S] rope'd K^T
                v_tiles = {}   # batch -> 16 x [128, 128] V tiles
                q1_tiles = {}  # macro -> head-1 q^T (kept for P1b)

                def attention(m, h, q_tile):
                    """Causal attention for macro m, local head h."""
                    bat, j = divmod(m, 4)
                    n_t = 4 * j + 4 if causal else 16
                    vo_ps = vo_pool.tile([128, RPC], F32, tag="vo",
                                         name=f"vo_{m}_{h}")
                    acc = fin_pool.tile([128, RPC], F32R, tag="acc",
                                        name=f"acc_{m}_{h}")
                    for t in range(n_t):
                        r0 = max(0, 128 * t - 512 * j) if causal else 0
                        lp = lg_pool.tile([128, RPC], F32, tag="lp",
                                          name=f"lp_{m}_{h}_{t}")
                        nc.tensor.matmul(
                            lp[:, r0:RPC],
                            ktr[bat][:, 128 * t : 128 * t + 128],
                            q_tile[:, r0:RPC],
                            start=True, stop=True,
                        )
                        if causal and t >= 4 * j:
                            nc.vector.tensor_add(
                                lp[:, r0 : r0 + 128],
                                lp[:, r0 : r0 + 128],
                                mask_t[:],
                            )
                        e = ex_pool.tile([128, RPC], F32R, tag="e",
                                         name=f"e_{m}_{h}_{t}")
                        nc.scalar.activation(
                            e[:, r0:RPC], lp[:, r0:RPC],
                            mybir.ActivationFunctionType.Exp,
                            scale=SCALE,
                        )
                        if t == 0:
                            nc.gpsimd.tensor_copy(acc[:], e[:])
                        elif t % 2 == 1:
                            nc.gpsimd.tensor_add(
                                acc[:, r0:RPC], acc[:, r0:RPC], e[:, r0:RPC]
                            )
                        else:
                            nc.vector.tensor_add(
                                acc[:, r0:RPC], acc[:, r0:RPC], e[:, r0:RPC]
                            )
                        nc.tensor.matmul(
                            vo_ps[:, r0:RPC],
                            v_tiles[bat][t][:],
                            e[:, r0:RPC],
                            start=(t == 0),
                            stop=(t == n_t - 1),
                        )
                    # softmax denominators; normalize; ship to A2A buffer
                    sp = sm_pool.tile([1, RPC], F32, tag="sp",
                                      name=f"sp_{m}_{h}")
                    nc.tensor.matmul(sp[:], onc_t[:], acc[:],
                                     start=True, stop=True)
                    rc = fin_pool.tile([1, RPC], F32R, tag="rc",
                                       name=f"rc_{m}_{h}")
                    nc.vector.reciprocal(rc[:], sp[:])
                    bc = aux_pool.tile([128, RPC], F32, tag="aux",
                                       name=f"bc_{m}_{h}")
                    nc.tensor.matmul(bc[:], onr_t[:], rc[:],
                                     start=True, stop=True)
                    rcb = fin_pool.tile([128, RPC], F32, tag="rcb",
                                        name=f"rcb_{m}_{h}")
                    nc.scalar.copy(rcb[:], bc[:])
                    voT = fin_pool.tile([128, RPC], F32R, tag="voT",
                                        name=f"voT_{m}_{h}")
                    nc.vector.tensor_mul(voT[:], vo_ps[:], rcb[:])
                    nc.sync.dma_start(
                        a2a_in[h][128 * m : 128 * m + 128, :], voT[:]
                    )

                # ---- P1a: proj + rope + head-0 attention ----
                for m in range(N_CORES):
                    bat, j = divmod(m, 4)
                    r0_glob = m * RPC
                    if j == 0:
                        ktr[bat] = kv_pool.tile([128, S], F32R,
                                                tag=f"ktr{bat}",
                                                name=f"ktr_{bat}")
                        v_tiles[bat] = [
                            kv_pool.tile([128, 128], F32R, tag=f"v{bat}_{i}",
                                         name=f"v_{bat}_{i}")
                            for i in range(16)
                        ]

                    # projection: z^T tiles for this macro
                    xts = []
                    for k in range(KT):
                        xt = xs_pool.tile([128, RPC], F32R, tag=f"x{k}",
                                          name=f"xt_{m}_{k}")
                        nc.sync.dma_start(
                            xt[:], xT[ts(k, 128), r0_glob : r0_glob + RPC]
                        )
                        xts.append(xt)
                    z = []
                    for M in range(4):
                        zp = zp_pool.tile([128, RPC], F32, tag="zp",
                                          name=f"zp_{m}_{M}")
                        for k in range(KT):
                            nc.tensor.matmul(
                                zp[:],
                                wq_tiles[k][:, ts(M, 128)],
                                xts[k][:],
                                start=(k == 0),
                                stop=(k == KT - 1),
                            )
                        zt = z_pool.tile([128, RPC], F32, tag=f"z{M}",
                                         name=f"z_{m}_{M}")
                        nc.scalar.activation(
                            zt[:], zp[:],
                            mybir.ActivationFunctionType.Identity,
                            bias=bias_t[:, M : M + 1],
                        )
                        z.append(zt)

                    # rope
                    sj = slice(512 * j, 512 * j + 512)
                    cs, sn = cos_t[:, sj], sin_t[:, sj]

                    def rope_half(dst, src_f, src_s, c_ap, s_ap, sign_f, nm):
                        t1 = rt_pool.tile([64, RPC], F32, tag="r1",
                                          name=f"r1_{nm}")
                        t2 = rt_pool.tile([64, RPC], F32, tag="r2",
                                          name=f"r2_{nm}")
                        nc.vector.tensor_mul(t1[:], src_f, c_ap)
                        nc.vector.tensor_mul(t2[:], src_s, s_ap)
                        if sign_f:
                            nc.vector.tensor_sub(dst, t1[:], t2[:])
                        else:
                            nc.vector.tensor_add(dst, t1[:], t2[:])

                    q_tr = []
                    for h in range(2):
                        if h == 0:
                            if causal:
                                qt = qtr_pool.tile([128, RPC], F32R, tag="q0",
                                                   name=f"q0_{m}")
                            else:
                                qt = q1k_pool.tile([128, RPC], F32R,
                                                   tag=f"q0_{m}",
                                                   name=f"q0k_{m}")
                        else:
                            qt = q1k_pool.tile([128, RPC], F32R, tag=f"q1_{m}",
                                               name=f"q1_{m}")
                        hs = slice(64 * h, 64 * h + 64)
                        rope_half(qt[0:64, :], z[0][hs, :], z[1][hs, :],
                                  cs[hs, :], sn[hs, :], True, f"qf{m}{h}")
                        rope_half(qt[64:128, :], z[0][hs, :], z[1][hs, :],
                                  sn[hs, :], cs[hs, :], False, f"qs{m}{h}")
                        q_tr.append(qt)
                    q1_tiles[m] = q_tr[1]
                    rope_half(ktr[bat][0:64, sj], z[2][0:64, :], z[3][0:64, :],
                              cs[0:64, :], sn[0:64, :], True, f"kf{m}")
                    rope_half(ktr[bat][64:128, sj], z[2][0:64, :], z[3][0:64, :],
                              sn[0:64, :], cs[0:64, :], False, f"ks{m}")

                    # v transposes: z[2]/z[3] partitions 64:128 hold v halves
                    for t4 in range(4):
                        vp = aux_pool.tile([128, 512], F32, tag="aux",
                                           name=f"vp_{m}_{t4}")
                        cslice = slice(128 * t4, 128 * t4 + 128)
                        nc.tensor.transpose(
                            vp[:, 0:64], z[2][64:128, cslice], id_t[64:128, :]
                        )
                        nc.tensor.transpose(
                            vp[:, 64:128], z[3][64:128, cslice], id_t[64:128, :]
                        )
                        nc.scalar.copy(v_tiles[bat][4 * j + t4][:],
                                       vp[:, 0:128])

                    if causal:
                        attention(m, 0, q_tr[0])
                        q0_keep[m] = None
                    else:
                        # non-causal needs the whole batch's KV first
                        q0_keep[m] = q_tr[0]
                        if j == 3:
                            for mm in range(4 * bat, 4 * bat + 4):
                                attention(mm, 0, q0_keep[mm])
                                q0_keep[mm] = None

                # ---- A2A #0 || P1b: head-1 attention ----
                nc.gpsimd.collective_compute(
                    "AllToAll",
                    mybir.AluOpType.bypass,
                    replica_groups=[list(range(N_CORES))],
                    ins=[a2a_in[0].opt()],
                    outs=[a2a_out[0].opt()],
                )
                for m in range(N_CORES):
                    attention(m, 1, q1_tiles[m])
                nc.gpsimd.collective_compute(
                    "AllToAll",
                    mybir.AluOpType.bypass,
                    replica_groups=[list(range(N_CORES))],
                    ins=[a2a_in[1].opt()],
                    outs=[a2a_out[1].opt()],
                )

            # ---- P3: out-proj for my 512 rows (even heads, then odd) ----
            nc.sync.dma_start(bo_t[:], bo_b[:])
            with (
                tc.tile_pool(name="vt", bufs=1) as vt_pool,
                tc.tile_pool(name="wop", bufs=1) as wo_pool,
                tc.tile_pool(name="op", bufs=2, space="PSUM") as op_pool,
                tc.tile_pool(name="ot", bufs=3) as ot_pool,
            ):
                wo_big = []
                for oc in range(4):
                    wb = wo_pool.tile([128, KT * 512], F32R, tag=f"wo{oc}",
                                      name=f"wo_{oc}")
                    nc.sync.dma_start(
                        wb[:].rearrange("p (t n) -> p t n", t=KT),
                        wo.rearrange("(t p) n -> p t n", p=128)[
                            :, :, ts(oc, 512)
                        ],
                    )
                    wo_big.append(wb)
                # voT_full K-tiles: parity h from a2a_out[h]; source core r
                # holds head 2r+h = Wo row-tile 2r+h
                vot_t = {}
                for h in range(2):
                    for r in range(8):
                        vt = vt_pool.tile([128, RPC], F32R, tag=f"vt{h}_{r}",
                                          name=f"vt_{h}_{r}")
                        nc.sync.dma_start(vt[:], a2a_out[h][ts(r, 128), :])
                        vot_t[2 * r + h] = vt
                for oc in range(4):
                    ps = [op_pool.tile([128, 512], F32, tag=f"op{M}",
                                       name=f"op_{oc}_{M}")
                          for M in range(4)]
                    for i, kk in enumerate(
                        [2 * r for r in range(8)] + [2 * r + 1 for r in range(8)]
                    ):
                        for M in range(4):
                            nc.tensor.matmul(
                                ps[M][:],
                                vot_t[kk][:, ts(M, 128)],
                                wo_big[oc][:, 512 * kk : 512 * kk + 512],
                                start=(i == 0),
                                stop=(i == KT - 1),
                            )
                    for M in range(4):
                        ot = ot_pool.tile([128, 512], F32, tag="ot",
                                          name=f"ot_{oc}_{M}")
                        nc.vector.tensor_add(
                            ot[:], ps[M][:], bo_t[:, ts(oc, 512)]
                        )
                        nc.sync.dma_start(
                            out_ap[ts(M, 128), ts(oc, 512)], ot[:]
                        )

    _fix_multiwait(nc)
    return nc


def _host_prep(x, cos, sin, Wq, bq, Wkv, bkv, Wo, bo):
    """Build the per-core input maps (all host-side slicing/transposes)."""
    xT = np.ascontiguousarray(x.reshape(ROWS, DIM).T)
    cosT = np.ascontiguousarray(cos.reshape(S, HD // 2).T)   # [64, S]
    sinT = np.ascontiguousarray(sin.reshape(S, HD // 2).T)
    cos2 = np.concatenate([cosT, cosT], axis=0)              # [128, S]
    sin2 = np.concatenate([sinT, sinT], axis=0)

    ii = np.arange(128)
    mask_tri = np.where(ii[None, :] >= ii[:, None], 0.0, NEG).astype(np.float32)
    ident = np.concatenate([np.zeros((64, 64), np.float32),
                            np.eye(64, dtype=np.float32)], axis=0)
    ones_col = np.ones((128, 1), np.float32)
    ones_row = np.ones((1, 128), np.float32)
    bo_b = np.broadcast_to(bo[None, :], (128, DIM)).copy()

    Wk, Wv = Wkv[:, : N_KV_HEADS * HD], Wkv[:, N_KV_HEADS * HD :]
    bk, bv = bkv[: N_KV_HEADS * HD], bkv[N_KV_HEADS * HD :]

    in_maps = []
    for c in range(N_CORES):
        h0, h1 = 2 * c, 2 * c + 1
        g = c // 2
        cols = [
            np.concatenate([Wq[:, h0 * HD : h0 * HD + 64],
                            Wq[:, h1 * HD : h1 * HD + 64]], axis=1),
            np.concatenate([Wq[:, h0 * HD + 64 : h0 * HD + 128],
                            Wq[:, h1 * HD + 64 : h1 * HD + 128]], axis=1),
            np.concatenate([Wk[:, g * HD : g * HD + 64],
                            Wv[:, g * HD : g * HD + 64]], axis=1),
            np.concatenate([Wk[:, g * HD + 64 : g * HD + 128],
                            Wv[:, g * HD + 64 : g * HD + 128]], axis=1),
        ]
        wqkv_c = np.ascontiguousarray(np.concatenate(cols, axis=1))
        bias_cols = [
            np.concatenate([bq[h0 * HD : h0 * HD + 64],
                            bq[h1 * HD : h1 * HD + 64]]),
            np.concatenate([bq[h0 * HD + 64 : h0 * HD + 128],
                            bq[h1 * HD + 64 : h1 * HD + 128]]),
            np.concatenate([bk[g * HD : g * HD + 64],
                            bv[g * HD : g * HD + 64]]),
            np.concatenate([bk[g * HD + 64 : g * HD + 128],
                            bv[g * HD + 64 : g * HD + 128]]),
        ]
        bias_mt = np.stack(bias_cols, axis=1).astype(np.float32)  # [128, 4]
        in_maps.append({
            "xT": xT, "wqkv": wqkv_c, "bias_mt": bias_mt,
            "cos2": cos2, "sin2": sin2, "mask_tri": mask_tri,
            "ident": ident, "ones_col": ones_col, "ones_row": ones_row,
            "wo": Wo, "bo_b": bo_b,
        })
    return in_maps


def kernel(x, cos, sin, Wq, bq, Wkv, bkv, Wo, bo, causal):
    from concourse.bass_utils import run_bass_kernel_spmd

    x = np.asarray(x, np.float32)
    cos = np.asarray(cos, np.float32)
    sin = np.asarray(sin, np.float32)
    Wq = np.asarray(Wq, np.float32)
    bq = np.asarray(bq, np.float32)
    Wkv = np.asarray(Wkv, np.float32)
    bkv = np.asarray(bkv, np.float32)
    Wo = np.asarray(Wo, np.float32)
    bo = np.asarray(bo, np.float32)
    causal = bool(np.asarray(causal).item())

    if causal not in _cache:
        _cache[causal] = _build(causal)
    nc = _cache[causal]

    in_maps = _host_prep(x, cos, sin, Wq, bq, Wkv, bkv, Wo, bo)
    res = run_bass_kernel_spmd(nc, in_maps, list(range(N_CORES)))
    out = np.concatenate([res.results[c]["out"] for c in range(N_CORES)], axis=0)
    return out.reshape(B, S, DIM)


# revision 22
# speedup vs baseline: 1.0098x; 1.0010x over previous
"""Multi-head self-attention (GQA + RoPE, causal) on 8 Trainium2 cores.

Sharding: tensor-parallel across heads for QKV projection + attention
(each core owns 2 q-heads and their kv-head), then AllToAlls reshard
the attention output from head-shards to sequence-shards, and each core
computes the output projection for its 512 rows against the full Wo.
Host concatenates the row shards - no AllReduce anywhere.

The attention is split by head so communication overlaps compute:
  P1a: proj + rope + head-0 attention for all 8 row-macros
  A2A#0 (head-0 vo shards)  ||  P1b: head-1 attention
  A2A#1 (head-1 vo shards)  ||  P3a: out-proj partial over even heads
  P3b: accumulate odd heads + bias, write row shard

All matmuls run as float32r (full PE speed at N>=512, ~1e-4 rel error).
Attention is computed in transposed layout (logits^T = k^T-tiles @ q^T)
so no score transposes are needed; softmax denominators accumulate on
DVE/GpSimd and reduce across partitions with a ones-column matmul.
"""

import numpy as np

N_CORES = 8
B, S, DIM = 2, 2048, 2048
N_HEADS, N_KV_HEADS, HD = 16, 4, 128
ROWS = B * S                     # 4096
RPC = ROWS // N_CORES            # 512 rows per core / per macro
KT = DIM // 128                  # 16 K tiles for the projections
SCALE = float(1.0 / np.sqrt(HD))
NEG = -60000.0                   # pre-scale masked logit; exp(SCALE*NEG) == 0

_cache = {}


def _fix_multiwait(nc):
    """Split >capacity sync waits (this walrus allows 1/inst, 2/EventSem)."""
    import bass_rust

    n = 0
    for f in nc.m.functions:
        for bb in f.blocks:
            insts = bb.instructions
            new_list = []
            changed = False
            for inst in insts:
                si = inst.sync_info
                cap = 2 if isinstance(inst, bass_rust.InstEventSemaphore) else 1
                if si is not None and len(si.on_wait) > cap:
                    waits = list(si.on_wait)
                    keep, extra = waits[:cap], waits[cap:]
                    for j in range(0, len(extra), 2):
                        es = bass_rust.InstEventSemaphore(
                            engine=inst.engine, name=f"waitfix_{n}"
                        )
                        es.sync_info = bass_rust.SyncInfo(
                            on_wait=extra[j : j + 2], on_update=[]
                        )
                        nc.register_instruction(es)
                        new_list.append(es)
                        n += 1
                    inst.sync_info = bass_rust.SyncInfo(
                        on_wait=keep, on_update=list(si.on_update)
                    )
                    changed = True
                new_list.append(inst)
            if changed:
                insts[:] = new_list
    return n


def _build(causal):
    import concourse.bass as bass
    import concourse.tile as tile
    from concourse import mybir
    from concourse.bass import ts

    F32R = mybir.dt.float32r
    F32 = mybir.dt.float32

    nc = bass.Bass("TRN2", target_bir_lowering=False, debug=False,
                   num_devices=N_CORES)

    # --- DRAM I/O (per core) ---
    xT = nc.dram_tensor("xT", [DIM, ROWS], F32R, kind="ExternalInput").ap()
    wqkv = nc.dram_tensor("wqkv", [DIM, 512], F32R, kind="ExternalInput").ap()
    bias_mt = nc.dram_tensor("bias_mt", [128, 4], F32R, kind="ExternalInput").ap()
    cos2 = nc.dram_tensor("cos2", [128, S], F32R, kind="ExternalInput").ap()
    sin2 = nc.dram_tensor("sin2", [128, S], F32R, kind="ExternalInput").ap()
    mask_tri = nc.dram_tensor("mask_tri", [128, 128], F32, kind="ExternalInput").ap()
    ident = nc.dram_tensor("ident", [128, 64], F32, kind="ExternalInput").ap()
    ones_col = nc.dram_tensor("ones_col", [128, 1], F32R, kind="ExternalInput").ap()
    ones_row = nc.dram_tensor("ones_row", [1, 128], F32R, kind="ExternalInput").ap()
    wo = nc.dram_tensor("wo", [DIM, DIM], F32R, kind="ExternalInput").ap()
    bo_b = nc.dram_tensor("bo_b", [128, DIM], F32, kind="ExternalInput").ap()
    out_ap = nc.dram_tensor("out", [RPC, DIM], F32, kind="ExternalOutput").ap()

    with tile.TileContext(nc) as tc:
        with (
            nc.allow_low_precision(reason="f32r attention kernel"),
            tc.tile_pool(name="dram", bufs=1, space="DRAM") as dram,
            tc.tile_pool(name="consts", bufs=1) as consts,
        ):
            # per-head A2A buffers: chunk m = [128, 512] voT of macro m
            a2a_in = [dram.tile([N_CORES * 128, RPC], F32R, name=f"a2ai{h}")
                      for h in range(2)]
            a2a_out = [dram.tile([N_CORES * 128, RPC], F32R, name=f"a2ao{h}")
                       for h in range(2)]

            # --- constants ---
            bias_t = consts.tile([128, 4], F32R, tag="bias")
            nc.sync.dma_start(bias_t[:], bias_mt[:])
            cos_t = consts.tile([128, S], F32R, tag="cos")
            sin_t = consts.tile([128, S], F32R, tag="sin")
            nc.sync.dma_start(cos_t[:], cos2[:])
            nc.sync.dma_start(sin_t[:], sin2[:])
            mask_t = consts.tile([128, 128], F32, tag="mask")
            nc.sync.dma_start(mask_t[:], mask_tri[:])
            id_t = consts.tile([128, 64], F32, tag="ident")
            nc.sync.dma_start(id_t[:], ident[:])
            onc_t = consts.tile([128, 1], F32R, tag="onc")
            nc.sync.dma_start(onc_t[:], ones_col[:])
            onr_t = consts.tile([1, 128], F32R, tag="onr")
            nc.sync.dma_start(onr_t[:], ones_row[:])
            bo_t = consts.tile([128, DIM], F32, tag="bo")

            with (
                tc.tile_pool(name="wqp", bufs=1) as wq_pool,
                tc.tile_pool(name="xs", bufs=1) as xs_pool,
                tc.tile_pool(name="zp", bufs=2, space="PSUM") as zp_pool,
                tc.tile_pool(name="z", bufs=2) as z_pool,
                tc.tile_pool(name="rt", bufs=4) as rt_pool,
                tc.tile_pool(name="qtr", bufs=2) as qtr_pool,
                tc.tile_pool(name="q1k", bufs=1) as q1k_pool,
                tc.tile_pool(name="kv", bufs=1) as kv_pool,
                tc.tile_pool(name="aux", bufs=1, space="PSUM") as aux_pool,
                tc.tile_pool(name="lg", bufs=3, space="PSUM") as lg_pool,
                tc.tile_pool(name="vo", bufs=2, space="PSUM") as vo_pool,
                tc.tile_pool(name="ex", bufs=3) as ex_pool,
                tc.tile_pool(name="fin", bufs=2) as fin_pool,
            ):
                wq_tiles = []
                for k in range(KT):
                    wk = wq_pool.tile([128, 512], F32R, tag=f"wq{k}",
                                      name=f"wq_{k}")
                    nc.sync.dma_start(wk[:], wqkv[ts(k, 128), :])
                    wq_tiles.append(wk)

                q0_keep = {}
                ktr = {}       # batch -> [128, S] rope'd K^T
                v_tiles = {}   # batch -> 16 x [128, 128] V tiles
                q1_tiles = {}  # macro -> head-1 q^T (kept for P1b)

                def attention(m, h, q_tile):
                    """Causal attention for macro m, local head h."""
                    bat, j = divmod(m, 4)
                    n_t = 4 * j + 4 if causal else 16
                    vo_ps = vo_pool.tile([128, RPC], F32, tag="vo",
                                         name=f"vo_{m}_{h}")
                    acc = fin_pool.tile([128, RPC], F32R, tag="acc",
                                        name=f"acc_{m}_{h}")
                    for t in range(n_t):
                        r0 = max(0, 128 * t - 512 * j) if causal else 0
                        lp = lg_pool.tile([128, RPC], F32, tag="lp",
                                          name=f"lp_{m}_{h}_{t}")
                        nc.tensor.matmul(
                            lp[:, r0:RPC],
                            ktr[bat][:, 128 * t : 128 * t + 128],
                            q_tile[:, r0:RPC],
                            start=True, stop=True,
                        )
                        if causal and t >= 4 * j:
                            nc.vector.tensor_add(
                                lp[:, r0 : r0 + 128],
                                lp[:, r0 : r0 + 128],
                                mask_t[:],
                            )
                        e = ex_pool.tile([128, RPC], F32R, tag="e",
                                         name=f"e_{m}_{h}_{t}")
                        nc.scalar.activation(
                            e[:, r0:RPC], lp[:, r0:RPC],
                            mybir.ActivationFunctionType.Exp,
                            scale=SCALE,
                        )
                        if t == 0:
                            nc.gpsimd.tensor_copy(acc[:], e[:])
                        elif t % 2 == 1:
                            nc.gpsimd.tensor_add(
                                acc[:, r0:RPC], acc[:, r0:RPC], e[:, r0:RPC]
                            )
                        else:
                            nc.vector.tensor_add(
                                acc[:, r0:RPC], acc[:, r0:RPC], e[:, r0:RPC]
                            )
                        nc.tensor.matmul(
                            vo_ps[:, r0:RPC],
                            v_tiles[bat][t][:],
                            e[:, r0:RPC],
                            start=(t == 0),
                            stop=(t == n_t - 1),
                        )
                    # softmax denominators; normalize; ship to A2A buffer
                    sp = aux_pool.tile([1, RPC], F32, tag="aux",
                                       name=f"sp_{m}_{h}")
                    nc.tensor.matmul(sp[:], onc_t[:], acc[:],
                                     start=True, stop=True)
                    rc = fin_pool.tile([1, RPC], F32R, tag="rc",
                                       name=f"rc_{m}_{h}")
                    nc.vector.reciprocal(rc[:], sp[:])
                    bc = aux_pool.tile([128, RPC], F32, tag="aux",
                                       name=f"bc_{m}_{h}")
                    nc.tensor.matmul(bc[:], onr_t[:], rc[:],
                                     start=True, stop=True)
                    rcb = fin_pool.tile([128, RPC], F32, tag="rcb",
                                        name=f"rcb_{m}_{h}")
                    nc.scalar.copy(rcb[:], bc[:])
                    voT = fin_pool.tile([128, RPC], F32R, tag="voT",
                                        name=f"voT_{m}_{h}")
                    nc.vector.tensor_mul(voT[:], vo_ps[:], rcb[:])
                    nc.sync.dma_start(
                        a2a_in[h][128 * m : 128 * m + 128, :], voT[:]
                    )

                # ---- P1a: proj + rope + head-0 attention ----
                for m in range(N_CORES):
                    bat, j = divmod(m, 4)
                    r0_glob = m * RPC
                    if j == 0:
                        ktr[bat] = kv_pool.tile([128, S], F32R,
                                                tag=f"ktr{bat}",
                                                name=f"ktr_{bat}")
                        v_tiles[bat] = [
                            kv_pool.tile([128, 128], F32R, tag=f"v{bat}_{i}",
                                         name=f"v_{bat}_{i}")
                            for i in range(16)
                        ]

                    # projection: z^T tiles for this macro
                    xts = []
                    for k in range(KT):
                        xt = xs_pool.tile([128, RPC], F32R, tag=f"x{k}",
                                          name=f"xt_{m}_{k}")
                        nc.sync.dma_start(
                            xt[:], xT[ts(k, 128), r0_glob : r0_glob + RPC]
                        )
                        xts.append(xt)
                    z = []
                    for M in range(4):
                        zp = zp_pool.tile([128, RPC], F32, tag="zp",
                                          name=f"zp_{m}_{M}")
                        for k in range(KT):
                            nc.tensor.matmul(
                                zp[:],
                                wq_tiles[k][:, ts(M, 128)],
                                xts[k][:],
                                start=(k == 0),
                                stop=(k == KT - 1),
                            )
                        zt = z_pool.tile([128, RPC], F32, tag=f"z{M}",
                                         name=f"z_{m}_{M}")
                        nc.scalar.activation(
                            zt[:], zp[:],
                            mybir.ActivationFunctionType.Identity,
                            bias=bias_t[:, M : M + 1],
                        )
                        z.append(zt)

                    # rope
                    sj = slice(512 * j, 512 * j + 512)
                    cs, sn = cos_t[:, sj], sin_t[:, sj]

                    def rope_half(dst, src_f, src_s, c_ap, s_ap, sign_f, nm):
                        t1 = rt_pool.tile([64, RPC], F32, tag="r1",
                                          name=f"r1_{nm}")
                        t2 = rt_pool.tile([64, RPC], F32, tag="r2",
                                          name=f"r2_{nm}")
                        nc.vector.tensor_mul(t1[:], src_f, c_ap)
                        nc.vector.tensor_mul(t2[:], src_s, s_ap)
                        if sign_f:
                            nc.vector.tensor_sub(dst, t1[:], t2[:])
                        else:
                            nc.vector.tensor_add(dst, t1[:], t2[:])

                    q_tr = []
                    for h in range(2):
                        if h == 0:
                            if causal:
                                qt = qtr_pool.tile([128, RPC], F32R, tag="q0",
                                                   name=f"q0_{m}")
                            else:
                                qt = q1k_pool.tile([128, RPC], F32R,
                                                   tag=f"q0_{m}",
                                                   name=f"q0k_{m}")
                        else:
                            qt = q1k_pool.tile([128, RPC], F32R, tag=f"q1_{m}",
                                               name=f"q1_{m}")
                        hs = slice(64 * h, 64 * h + 64)
                        rope_half(qt[0:64, :], z[0][hs, :], z[1][hs, :],
                                  cs[hs, :], sn[hs, :], True, f"qf{m}{h}")
                        rope_half(qt[64:128, :], z[0][hs, :], z[1][hs, :],
                                  sn[hs, :], cs[hs, :], False, f"qs{m}{h}")
                        q_tr.append(qt)
                    q1_tiles[m] = q_tr[1]
                    rope_half(ktr[bat][0:64, sj], z[2][0:64, :], z[3][0:64, :],
                              cs[0:64, :], sn[0:64, :], True, f"kf{m}")
                    rope_half(ktr[bat][64:128, sj], z[2][0:64, :], z[3][0:64, :],
                              sn[0:64, :], cs[0:64, :], False, f"ks{m}")

                    # v transposes: z[2]/z[3] partitions 64:128 hold v halves
                    for t4 in range(4):
                        vp = aux_pool.tile([128, 512], F32, tag="aux",
                                           name=f"vp_{m}_{t4}")
                        cslice = slice(128 * t4, 128 * t4 + 128)
                        nc.tensor.transpose(
                            vp[:, 0:64], z[2][64:128, cslice], id_t[64:128, :]
                        )
                        nc.tensor.transpose(
                            vp[:, 64:128], z[3][64:128, cslice], id_t[64:128, :]
                        )
                        nc.scalar.copy(v_tiles[bat][4 * j + t4][:],
                                       vp[:, 0:128])

                    if causal:
                        attention(m, 0, q_tr[0])
                        q0_keep[m] = None
                    else:
                        # non-causal needs the whole batch's KV first
                        q0_keep[m] = q_tr[0]
                        if j == 3:
                            for mm in range(4 * bat, 4 * bat + 4):
                                attention(mm, 0, q0_keep[mm])
                                q0_keep[mm] = None

                # ---- A2A #0 || P1b: head-1 attention ----
                nc.gpsimd.collective_compute(
                    "AllToAll",
                    mybir.AluOpType.bypass,
                    replica_groups=[list(range(N_CORES))],
                    ins=[a2a_in[0].opt()],
                    outs=[a2a_out[0].opt()],
                )
                for m in range(N_CORES):
                    attention(m, 1, q1_tiles[m])
                nc.gpsimd.collective_compute(
                    "AllToAll",
                    mybir.AluOpType.bypass,
                    replica_groups=[list(range(N_CORES))],
                    ins=[a2a_in[1].opt()],
                    outs=[a2a_out[1].opt()],
                )

            # ---- P3: out-proj for my 512 rows (even heads, then odd) ----
            nc.sync.dma_start(bo_t[:], bo_b[:])
            with (
                tc.tile_pool(name="vt", bufs=1) as vt_pool,
                tc.tile_pool(name="wop", bufs=1) as wo_pool,
                tc.tile_pool(name="op", bufs=2, space="PSUM") as op_pool,
                tc.tile_pool(name="ot", bufs=3) as ot_pool,
            ):
                wo_big = []
                for oc in range(4):
                    wb = wo_pool.tile([128, KT * 512], F32R, tag=f"wo{oc}",
                                      name=f"wo_{oc}")
                    nc.sync.dma_start(
                        wb[:].rearrange("p (t n) -> p t n", t=KT),
                        wo.rearrange("(t p) n -> p t n", p=128)[
                            :, :, ts(oc, 512)
                        ],
                    )
                    wo_big.append(wb)
                # voT_full K-tiles: parity h from a2a_out[h]; source core r
                # holds head 2r+h = Wo row-tile 2r+h
                vot_t = {}
                for h in range(2):
                    for r in range(8):
                        vt = vt_pool.tile([128, RPC], F32R, tag=f"vt{h}_{r}",
                                          name=f"vt_{h}_{r}")
                        nc.sync.dma_start(vt[:], a2a_out[h][ts(r, 128), :])
                        vot_t[2 * r + h] = vt
                for oc in range(4):
                    ps = [op_pool.tile([128, 512], F32, tag=f"op{M}",
                                       name=f"op_{oc}_{M}")
                          for M in range(4)]
                    for i, kk in enumerate(
                        [2 * r for r in range(8)] + [2 * r + 1 for r in range(8)]
                    ):
                        for M in range(4):
                            nc.tensor.matmul(
                                ps[M][:],
                                vot_t[kk][:, ts(M, 128)],
                                wo_big[oc][:, 512 * kk : 512 * kk + 512],
                                start=(i == 0),
                                stop=(i == KT - 1),
                            )
                    for M in range(4):
                        ot = ot_pool.tile([128, 512], F32, tag="ot",
                                          name=f"ot_{oc}_{M}")
                        nc.vector.tensor_add(
                            ot[:], ps[M][:], bo_t[:, ts(oc, 512)]
                        )
                        nc.sync.dma_start(
                            out_ap[ts(M, 128), ts(oc, 512)], ot[:]
                        )

    _fix_multiwait(nc)
    return nc


def _host_prep(x, cos, sin, Wq, bq, Wkv, bkv, Wo, bo):
    """Build the per-core input maps (all host-side slicing/transposes)."""
    xT = np.ascontiguousarray(x.reshape(ROWS, DIM).T)
    cosT = np.ascontiguousarray(cos.reshape(S, HD // 2).T)   # [64, S]
    sinT = np.ascontiguousarray(sin.reshape(S, HD // 2).T)
    cos2 = np.concatenate([cosT, cosT], axis=0)              # [128, S]
    sin2 = np.concatenate([sinT, sinT], axis=0)

    ii = np.arange(128)
    mask_tri = np.where(ii[None, :] >= ii[:, None], 0.0, NEG).astype(np.float32)
    ident = np.concatenate([np.zeros((64, 64), np.float32),
                            np.eye(64, dtype=np.float32)], axis=0)
    ones_col = np.ones((128, 1), np.float32)
    ones_row = np.ones((1, 128), np.float32)
    bo_b = np.broadcast_to(bo[None, :], (128, DIM)).copy()

    Wk, Wv = Wkv[:, : N_KV_HEADS * HD], Wkv[:, N_KV_HEADS * HD :]
    bk, bv = bkv[: N_KV_HEADS * HD], bkv[N_KV_HEADS * HD :]

    in_maps = []
    for c in range(N_CORES):
        h0, h1 = 2 * c, 2 * c + 1
        g = c // 2
        cols = [
            np.concatenate([Wq[:, h0 * HD : h0 * HD + 64],
                            Wq[:, h1 * HD : h1 * HD + 64]], axis=1),
            np.concatenate([Wq[:, h0 * HD + 64 : h0 * HD + 128],
                            Wq[:, h1 * HD + 64 : h1 * HD + 128]], axis=1),
            np.concatenate([Wk[:, g * HD : g * HD + 64],
                            Wv[:, g * HD : g * HD + 64]], axis=1),
            np.concatenate([Wk[:, g * HD + 64 : g * HD + 128],
                            Wv[:, g * HD + 64 : g * HD + 128]], axis=1),
        ]
        wqkv_c = np.ascontiguousarray(np.concatenate(cols, axis=1))
        bias_cols = [
            np.concatenate([bq[h0 * HD : h0 * HD + 64],
                            bq[h1 * HD : h1 * HD + 64]]),
            np.concatenate([bq[h0 * HD + 64 : h0 * HD + 128],
                            bq[h1 * HD + 64 : h1 * HD + 128]]),
            np.concatenate([bk[g * HD : g * HD + 64],
                            bv[g * HD : g * HD + 64]]),
            np.concatenate([bk[g * HD + 64 : g * HD + 128],
                            bv[g * HD + 64 : g * HD + 128]]),
        ]
        bias_mt = np.stack(bias_cols, axis=1).astype(np.float32)  # [128, 4]
        in_maps.append({
            "xT": xT, "wqkv": wqkv_c, "bias_mt": bias_mt,
            "cos2": cos2, "sin2": sin2, "mask_tri": mask_tri,
            "ident": ident, "ones_col": ones_col, "ones_row": ones_row,
            "wo": Wo, "bo_b": bo_b,
        })
    return in_maps


def kernel(x, cos, sin, Wq, bq, Wkv, bkv, Wo, bo, causal):
    from concourse.bass_utils import run_bass_kernel_spmd

    x = np.asarray(x, np.float32)
    cos = np.asarray(cos, np.float32)
    sin = np.asarray(sin, np.float32)
    Wq = np.asarray(Wq, np.float32)
    bq = np.asarray(bq, np.float32)
    Wkv = np.asarray(Wkv, np.float32)
    bkv = np.asarray(bkv, np.float32)
    Wo = np.asarray(Wo, np.float32)
    bo = np.asarray(bo, np.float32)
    causal = bool(np.asarray(causal).item())

    if causal not in _cache:
        _cache[causal] = _build(causal)
    nc = _cache[causal]

    in_maps = _host_prep(x, cos, sin, Wq, bq, Wkv, bkv, Wo, bo)
    res = run_bass_kernel_spmd(nc, in_maps, list(range(N_CORES)))
    out = np.concatenate([res.results[c]["out"] for c in range(N_CORES)], axis=0)
    return out.reshape(B, S, DIM)


# revision 33
# speedup vs baseline: 1.0304x; 1.0204x over previous
"""Multi-head self-attention (GQA + RoPE, causal) on 8 Trainium2 cores.

Sharding: tensor-parallel across heads for QKV projection + attention
(each core owns 2 q-heads and their kv-head), then AllToAlls reshard
the attention output from head-shards to sequence-shards, and each core
computes the output projection for its 512 rows against the full Wo.
Host concatenates the row shards - no AllReduce anywhere.

The attention is split by head so communication overlaps compute:
  P1a: proj + rope + head-0 attention for all 8 row-macros
  A2A#0 (head-0 vo shards)  ||  P1b: head-1 attention
  A2A#1 (head-1 vo shards)  ||  P3a: out-proj partial over even heads
  P3b: accumulate odd heads + bias, write row shard

All matmuls run as float32r (full PE speed at N>=512, ~1e-4 rel error).
Attention is computed in transposed layout (logits^T = k^T-tiles @ q^T)
so no score transposes are needed; softmax denominators accumulate on
DVE/GpSimd and reduce across partitions with a ones-column matmul.
"""

import numpy as np

N_CORES = 8
B, S, DIM = 2, 2048, 2048
N_HEADS, N_KV_HEADS, HD = 16, 4, 128
ROWS = B * S                     # 4096
RPC = ROWS // N_CORES            # 512 rows per core / per macro
KT = DIM // 128                  # 16 K tiles for the projections
SCALE = float(1.0 / np.sqrt(HD))
NEG = -60000.0                   # pre-scale masked logit; exp(SCALE*NEG) == 0

_cache = {}


def _fix_multiwait(nc):
    """Split >capacity sync waits (this walrus allows 1/inst, 2/EventSem)."""
    import bass_rust

    n = 0
    for f in nc.m.functions:
        for bb in f.blocks:
            insts = bb.instructions
            new_list = []
            changed = False
            for inst in insts:
                si = inst.sync_info
                cap = 2 if isinstance(inst, bass_rust.InstEventSemaphore) else 1
                if si is not None and len(si.on_wait) > cap:
                    waits = list(si.on_wait)
                    keep, extra = waits[:cap], waits[cap:]
                    for j in range(0, len(extra), 2):
                        es = bass_rust.InstEventSemaphore(
                            engine=inst.engine, name=f"waitfix_{n}"
                        )
                        es.sync_info = bass_rust.SyncInfo(
                            on_wait=extra[j : j + 2], on_update=[]
                        )
                        nc.register_instruction(es)
                        new_list.append(es)
                        n += 1
                    inst.sync_info = bass_rust.SyncInfo(
                        on_wait=keep, on_update=list(si.on_update)
                    )
                    changed = True
                new_list.append(inst)
            if changed:
                insts[:] = new_list
    return n


def _build(causal):
    import concourse.bass as bass
    import concourse.tile as tile
    from concourse import mybir
    from concourse.bass import ts

    F32R = mybir.dt.float32r
    F32 = mybir.dt.float32

    nc = bass.Bass("TRN2", target_bir_lowering=False, debug=False,
                   num_devices=N_CORES)

    # --- DRAM I/O (per core) ---
    xT = nc.dram_tensor("xT", [DIM, ROWS], F32R, kind="ExternalInput").ap()
    wqkv = nc.dram_tensor("wqkv", [DIM, 512], F32R, kind="ExternalInput").ap()
    bias_mt = nc.dram_tensor("bias_mt", [128, 4], F32R, kind="ExternalInput").ap()
    cos2 = nc.dram_tensor("cos2", [128, S], F32R, kind="ExternalInput").ap()
    sin2 = nc.dram_tensor("sin2", [128, S], F32R, kind="ExternalInput").ap()
    mask_tri = nc.dram_tensor("mask_tri", [128, 128], F32, kind="ExternalInput").ap()
    ident = nc.dram_tensor("ident", [128, 64], F32, kind="ExternalInput").ap()
    ones_col = nc.dram_tensor("ones_col", [128, 1], F32R, kind="ExternalInput").ap()
    ones_row = nc.dram_tensor("ones_row", [1, 128], F32R, kind="ExternalInput").ap()
    wo = nc.dram_tensor("wo", [DIM, DIM], F32R, kind="ExternalInput").ap()
    bo_b = nc.dram_tensor("bo_b", [128, DIM], F32, kind="ExternalInput").ap()
    out_ap = nc.dram_tensor("out", [RPC, DIM], F32, kind="ExternalOutput").ap()

    with tile.TileContext(nc) as tc:
        with (
            nc.allow_low_precision(reason="f32r attention kernel"),
            tc.tile_pool(name="dram", bufs=1, space="DRAM") as dram,
            tc.tile_pool(name="consts", bufs=1) as consts,
        ):
            # per-head A2A buffers: chunk m = [128, 512] voT of macro m
            a2a_in = [dram.tile([N_CORES * 128, RPC], F32R, name=f"a2ai{h}")
                      for h in range(2)]
            a2a_out = [dram.tile([N_CORES * 128, RPC], F32R, name=f"a2ao{h}")
                       for h in range(2)]

            # --- constants ---
            bias_t = consts.tile([128, 4], F32R, tag="bias")
            nc.sync.dma_start(bias_t[:], bias_mt[:])
            cos_t = consts.tile([128, S], F32R, tag="cos")
            sin_t = consts.tile([128, S], F32R, tag="sin")
            nc.sync.dma_start(cos_t[:], cos2[:])
            nc.sync.dma_start(sin_t[:], sin2[:])
            mask_t = consts.tile([128, 128], F32, tag="mask")
            nc.sync.dma_start(mask_t[:], mask_tri[:])
            id_t = consts.tile([128, 64], F32, tag="ident")
            nc.sync.dma_start(id_t[:], ident[:])
            onc_t = consts.tile([128, 1], F32R, tag="onc")
            nc.sync.dma_start(onc_t[:], ones_col[:])
            onr_t = consts.tile([1, 128], F32R, tag="onr")
            nc.sync.dma_start(onr_t[:], ones_row[:])
            bo_t = consts.tile([128, DIM], F32, tag="bo")

            with (
                tc.tile_pool(name="wqp", bufs=1) as wq_pool,
                tc.tile_pool(name="xs", bufs=1) as xs_pool,
                tc.tile_pool(name="zp", bufs=2, space="PSUM") as zp_pool,
                tc.tile_pool(name="z", bufs=2) as z_pool,
                tc.tile_pool(name="rt", bufs=4) as rt_pool,
                tc.tile_pool(name="qtr", bufs=2) as qtr_pool,
                tc.tile_pool(name="q1k", bufs=1) as q1k_pool,
                tc.tile_pool(name="kv", bufs=1) as kv_pool,
                tc.tile_pool(name="aux", bufs=1, space="PSUM") as aux_pool,
                tc.tile_pool(name="lg", bufs=3, space="PSUM") as lg_pool,
                tc.tile_pool(name="vo", bufs=2, space="PSUM") as vo_pool,
                tc.tile_pool(name="ex", bufs=4 if causal else 3) as ex_pool,
                tc.tile_pool(name="fin", bufs=3 if causal else 2) as fin_pool,
            ):
                wq_tiles = []
                for k in range(KT):
                    wk = wq_pool.tile([128, 512], F32R, tag=f"wq{k}",
                                      name=f"wq_{k}")
                    nc.sync.dma_start(wk[:], wqkv[ts(k, 128), :])
                    wq_tiles.append(wk)

                q0_keep = {}
                ktr = {}       # batch -> [128, S] rope'd K^T
                v_tiles = {}   # batch -> 16 x [128, 128] V tiles
                q1_tiles = {}  # macro -> head-1 q^T (kept for P1b)

                def attention(m, h, q_tile):
                    """Causal attention for macro m, local head h."""
                    bat, j = divmod(m, 4)
                    n_t = 4 * j + 4 if causal else 16
                    vo_ps = vo_pool.tile([128, RPC], F32, tag="vo",
                                         name=f"vo_{m}_{h}")
                    acc = fin_pool.tile([128, RPC], F32R, tag="acc",
                                        name=f"acc_{m}_{h}")
                    for t in range(n_t):
                        r0 = max(0, 128 * t - 512 * j) if causal else 0
                        lp = lg_pool.tile([128, RPC], F32, tag="lp",
                                          name=f"lp_{m}_{h}_{t}")
                        nc.tensor.matmul(
                            lp[:, r0:RPC],
                            ktr[bat][:, 128 * t : 128 * t + 128],
                            q_tile[:, r0:RPC],
                            start=True, stop=True,
                        )
                        if causal and t >= 4 * j:
                            nc.vector.tensor_add(
                                lp[:, r0 : r0 + 128],
                                lp[:, r0 : r0 + 128],
                                mask_t[:],
                            )
                        e = ex_pool.tile([128, RPC], F32R, tag="e",
                                         name=f"e_{m}_{h}_{t}")
                        nc.scalar.activation(
                            e[:, r0:RPC], lp[:, r0:RPC],
                            mybir.ActivationFunctionType.Exp,
                            scale=SCALE,
                        )
                        if t == 0:
                            nc.gpsimd.tensor_copy(acc[:], e[:])
                        elif t % 2 == 1:
                            nc.gpsimd.tensor_add(
                                acc[:, r0:RPC], acc[:, r0:RPC], e[:, r0:RPC]
                            )
                        else:
                            nc.vector.tensor_add(
                                acc[:, r0:RPC], acc[:, r0:RPC], e[:, r0:RPC]
                            )
                        nc.tensor.matmul(
                            vo_ps[:, r0:RPC],
                            v_tiles[bat][t][:],
                            e[:, r0:RPC],
                            start=(t == 0),
                            stop=(t == n_t - 1),
                        )
                    # softmax denominators; normalize; ship to A2A buffer
                    sp = aux_pool.tile([1, RPC], F32, tag="aux",
                                       name=f"sp_{m}_{h}")
                    nc.tensor.matmul(sp[:], onc_t[:], acc[:],
                                     start=True, stop=True)
                    rc = fin_pool.tile([1, RPC], F32R, tag="rc",
                                       name=f"rc_{m}_{h}")
                    nc.vector.reciprocal(rc[:], sp[:])
                    bc = aux_pool.tile([128, RPC], F32, tag="aux",
                                       name=f"bc_{m}_{h}")
                    nc.tensor.matmul(bc[:], onr_t[:], rc[:],
                                     start=True, stop=True)
                    rcb = fin_pool.tile([128, RPC], F32, tag="rcb",
                                        name=f"rcb_{m}_{h}")
                    nc.scalar.copy(rcb[:], bc[:])
                    voT = fin_pool.tile([128, RPC], F32R, tag="voT",
                                        name=f"voT_{m}_{h}")
                    nc.vector.tensor_mul(voT[:], vo_ps[:], rcb[:])
                    nc.sync.dma_start(
                        a2a_in[h][128 * m : 128 * m + 128, :], voT[:]
                    )

                # ---- P1a: proj + rope + head-0 attention ----
                for m in range(N_CORES):
                    bat, j = divmod(m, 4)
                    r0_glob = m * RPC
                    if j == 0:
                        ktr[bat] = kv_pool.tile([128, S], F32R,
                                                tag=f"ktr{bat}",
                                                name=f"ktr_{bat}")
                        v_tiles[bat] = [
                            kv_pool.tile([128, 128], F32R, tag=f"v{bat}_{i}",
                                         name=f"v_{bat}_{i}")
                            for i in range(16)
                        ]

                    # projection: z^T tiles for this macro
                    xts = []
                    for k in range(KT):
                        xt = xs_pool.tile([128, RPC], F32R, tag=f"x{k}",
                                          name=f"xt_{m}_{k}")
                        nc.sync.dma_start(
                            xt[:], xT[ts(k, 128), r0_glob : r0_glob + RPC]
                        )
                        xts.append(xt)
                    z = []
                    for M in range(4):
                        zp = zp_pool.tile([128, RPC], F32, tag="zp",
                                          name=f"zp_{m}_{M}")
                        for k in range(KT):
                            nc.tensor.matmul(
                                zp[:],
                                wq_tiles[k][:, ts(M, 128)],
                                xts[k][:],
                                start=(k == 0),
                                stop=(k == KT - 1),
                            )
                        zt = z_pool.tile([128, RPC], F32, tag=f"z{M}",
                                         name=f"z_{m}_{M}")
                        nc.scalar.activation(
                            zt[:], zp[:],
                            mybir.ActivationFunctionType.Identity,
                            bias=bias_t[:, M : M + 1],
                        )
                        z.append(zt)

                    # rope
                    sj = slice(512 * j, 512 * j + 512)
                    cs, sn = cos_t[:, sj], sin_t[:, sj]

                    def rope_half(dst, src_f, src_s, c_ap, s_ap, sign_f, nm):
                        t1 = rt_pool.tile([64, RPC], F32, tag="r1",
                                          name=f"r1_{nm}")
                        t2 = rt_pool.tile([64, RPC], F32, tag="r2",
                                          name=f"r2_{nm}")
                        nc.vector.tensor_mul(t1[:], src_f, c_ap)
                        nc.vector.tensor_mul(t2[:], src_s, s_ap)
                        if sign_f:
                            nc.vector.tensor_sub(dst, t1[:], t2[:])
                        else:
                            nc.vector.tensor_add(dst, t1[:], t2[:])

                    q_tr = []
                    for h in range(2):
                        if h == 0:
                            if causal:
                                qt = qtr_pool.tile([128, RPC], F32R, tag="q0",
                                                   name=f"q0_{m}")
                            else:
                                qt = q1k_pool.tile([128, RPC], F32R,
                                                   tag=f"q0_{m}",
                                                   name=f"q0k_{m}")
                        else:
                            qt = q1k_pool.tile([128, RPC], F32R, tag=f"q1_{m}",
                                               name=f"q1_{m}")
                        hs = slice(64 * h, 64 * h + 64)
                        rope_half(qt[0:64, :], z[0][hs, :], z[1][hs, :],
                                  cs[hs, :], sn[hs, :], True, f"qf{m}{h}")
                        rope_half(qt[64:128, :], z[0][hs, :], z[1][hs, :],
                                  sn[hs, :], cs[hs, :], False, f"qs{m}{h}")
                        q_tr.append(qt)
                    q1_tiles[m] = q_tr[1]
                    rope_half(ktr[bat][0:64, sj], z[2][0:64, :], z[3][0:64, :],
                              cs[0:64, :], sn[0:64, :], True, f"kf{m}")
                    rope_half(ktr[bat][64:128, sj], z[2][0:64, :], z[3][0:64, :],
                              sn[0:64, :], cs[0:64, :], False, f"ks{m}")

                    # v transposes: z[2]/z[3] partitions 64:128 hold v halves
                    for t4 in range(4):
                        vp = aux_pool.tile([128, 512], F32, tag="aux",
                                           name=f"vp_{m}_{t4}")
                        cslice = slice(128 * t4, 128 * t4 + 128)
                        nc.tensor.transpose(
                            vp[:, 0:64], z[2][64:128, cslice], id_t[64:128, :]
                        )
                        nc.tensor.transpose(
                            vp[:, 64:128], z[3][64:128, cslice], id_t[64:128, :]
                        )
                        nc.scalar.copy(v_tiles[bat][4 * j + t4][:],
                                       vp[:, 0:128])

                    if causal:
                        # software-pipeline: emit macro m-1's head-0
                        # attention after macro m's projection so PE fills
                        # exp/rope latency gaps with proj matmuls
                        q0_keep[m] = q_tr[0]
                        if m > 0:
                            attention(m - 1, 0, q0_keep.pop(m - 1))
                    else:
                        # non-causal needs the whole batch's KV first
                        q0_keep[m] = q_tr[0]
                        if j == 3:
                            for mm in range(4 * bat, 4 * bat + 4):
                                attention(mm, 0, q0_keep[mm])
                                q0_keep[mm] = None
                if causal:
                    attention(N_CORES - 1, 0, q0_keep.pop(N_CORES - 1))

                # ---- A2A #0 || P1b: head-1 attention ----
                nc.gpsimd.collective_compute(
                    "AllToAll",
                    mybir.AluOpType.bypass,
                    replica_groups=[list(range(N_CORES))],
                    ins=[a2a_in[0].opt()],
                    outs=[a2a_out[0].opt()],
                )
                for m in range(N_CORES):
                    attention(m, 1, q1_tiles[m])
                nc.gpsimd.collective_compute(
                    "AllToAll",
                    mybir.AluOpType.bypass,
                    replica_groups=[list(range(N_CORES))],
                    ins=[a2a_in[1].opt()],
                    outs=[a2a_out[1].opt()],
                )

            # ---- P3: out-proj for my 512 rows (even heads, then odd) ----
            nc.sync.dma_start(bo_t[:], bo_b[:])
            with (
                tc.tile_pool(name="vt", bufs=1) as vt_pool,
                tc.tile_pool(name="wop", bufs=1) as wo_pool,
                tc.tile_pool(name="op", bufs=2, space="PSUM") as op_pool,
                tc.tile_pool(name="ot", bufs=3) as ot_pool,
            ):
                wo_big = []
                for oc in range(4):
                    wb = wo_pool.tile([128, KT * 512], F32R, tag=f"wo{oc}",
                                      name=f"wo_{oc}")
                    nc.sync.dma_start(
                        wb[:].rearrange("p (t n) -> p t n", t=KT),
                        wo.rearrange("(t p) n -> p t n", p=128)[
                            :, :, ts(oc, 512)
                        ],
                    )
                    wo_big.append(wb)
                # voT_full K-tiles: parity h from a2a_out[h]; source core r
                # holds head 2r+h = Wo row-tile 2r+h
                vot_t = {}
                for h in range(2):
                    for r in range(8):
                        vt = vt_pool.tile([128, RPC], F32R, tag=f"vt{h}_{r}",
                                          name=f"vt_{h}_{r}")
                        nc.sync.dma_start(vt[:], a2a_out[h][ts(r, 128), :])
                        vot_t[2 * r + h] = vt
                for oc in range(4):
                    ps = [op_pool.tile([128, 512], F32, tag=f"op{M}",
                                       name=f"op_{oc}_{M}")
                          for M in range(4)]
                    for i, kk in enumerate(
                        [2 * r for r in range(8)] + [2 * r + 1 for r in range(8)]
                    ):
                        for M in range(4):
                            nc.tensor.matmul(
                                ps[M][:],
                                vot_t[kk][:, ts(M, 128)],
                                wo_big[oc][:, 512 * kk : 512 * kk + 512],
                                start=(i == 0),
                                stop=(i == KT - 1),
                            )
                    for M in range(4):
                        ot = ot_pool.tile([128, 512], F32, tag="ot",
                                          name=f"ot_{oc}_{M}")
                        nc.vector.tensor_add(
                            ot[:], ps[M][:], bo_t[:, ts(oc, 512)]
                        )
                        nc.sync.dma_start(
                            out_ap[ts(M, 128), ts(oc, 512)], ot[:]
                        )

    _fix_multiwait(nc)
    return nc


def _host_prep(x, cos, sin, Wq, bq, Wkv, bkv, Wo, bo):
    """Build the per-core input maps (all host-side slicing/transposes)."""
    xT = np.ascontiguousarray(x.reshape(ROWS, DIM).T)
    cosT = np.ascontiguousarray(cos.reshape(S, HD // 2).T)   # [64, S]
    sinT = np.ascontiguousarray(sin.reshape(S, HD // 2).T)
    cos2 = np.concatenate([cosT, cosT], axis=0)              # [128, S]
    sin2 = np.concatenate([sinT, sinT], axis=0)

    ii = np.arange(128)
    mask_tri = np.where(ii[None, :] >= ii[:, None], 0.0, NEG).astype(np.float32)
    ident = np.concatenate([np.zeros((64, 64), np.float32),
                            np.eye(64, dtype=np.float32)], axis=0)
    ones_col = np.ones((128, 1), np.float32)
    ones_row = np.ones((1, 128), np.float32)
    bo_b = np.broadcast_to(bo[None, :], (128, DIM)).copy()

    Wk, Wv = Wkv[:, : N_KV_HEADS * HD], Wkv[:, N_KV_HEADS * HD :]
    bk, bv = bkv[: N_KV_HEADS * HD], bkv[N_KV_HEADS * HD :]

    in_maps = []
    for c in range(N_CORES):
        h0, h1 = 2 * c, 2 * c + 1
        g = c // 2
        cols = [
            np.concatenate([Wq[:, h0 * HD : h0 * HD + 64],
                            Wq[:, h1 * HD : h1 * HD + 64]], axis=1),
            np.concatenate([Wq[:, h0 * HD + 64 : h0 * HD + 128],
                            Wq[:, h1 * HD + 64 : h1 * HD + 128]], axis=1),
            np.concatenate([Wk[:, g * HD : g * HD + 64],
                            Wv[:, g * HD : g * HD + 64]], axis=1),
            np.concatenate([Wk[:, g * HD + 64 : g * HD + 128],
                            Wv[:, g * HD + 64 : g * HD + 128]], axis=1),
        ]
        wqkv_c = np.ascontiguousarray(np.concatenate(cols, axis=1))
        bias_cols = [
            np.concatenate([bq[h0 * HD : h0 * HD + 64],
                            bq[h1 * HD : h1 * HD + 64]]),
            np.concatenate([bq[h0 * HD + 64 : h0 * HD + 128],
                            bq[h1 * HD + 64 : h1 * HD + 128]]),
            np.concatenate([bk[g * HD : g * HD + 64],
                            bv[g * HD : g * HD + 64]]),
            np.concatenate([bk[g * HD + 64 : g * HD + 128],
                            bv[g * HD + 64 : g * HD + 128]]),
        ]
        bias_mt = np.stack(bias_cols, axis=1).astype(np.float32)  # [128, 4]
        in_maps.append({
            "xT": xT, "wqkv": wqkv_c, "bias_mt": bias_mt,
            "cos2": cos2, "sin2": sin2, "mask_tri": mask_tri,
            "ident": ident, "ones_col": ones_col, "ones_row": ones_row,
            "wo": Wo, "bo_b": bo_b,
        })
    return in_maps


def kernel(x, cos, sin, Wq, bq, Wkv, bkv, Wo, bo, causal):
    from concourse.bass_utils import run_bass_kernel_spmd

    x = np.asarray(x, np.float32)
    cos = np.asarray(cos, np.float32)
    sin = np.asarray(sin, np.float32)
    Wq = np.asarray(Wq, np.float32)
    bq = np.asarray(bq, np.float32)
    Wkv = np.asarray(Wkv, np.float32)
    bkv = np.asarray(bkv, np.float32)
    Wo = np.asarray(Wo, np.float32)
    bo = np.asarray(bo, np.float32)
    causal = bool(np.asarray(causal).item())

    if causal not in _cache:
        _cache[causal] = _build(causal)
    nc = _cache[causal]

    in_maps = _host_prep(x, cos, sin, Wq, bq, Wkv, bkv, Wo, bo)
    res = run_bass_kernel_spmd(nc, in_maps, list(range(N_CORES)))
    out = np.concatenate([res.results[c]["out"] for c in range(N_CORES)], axis=0)
    return out.reshape(B, S, DIM)
